# revision 19
# baseline (speedup 1.0000x reference)
"""Trainium2 Bass kernel for a 12-layer prefix-causal transformer.

Sharding: data-parallel over batch B=8 across 8 NeuronCores (1 sequence per
core, weights replicated, no collectives).

v3 (from v2's fp8 DoubleRow attention baseline), trace-driven restructure:
  - scores exp merged into multi-bank PSUM groups: 6 big ACTIVATEs per
    head-pair instead of 24 small ones (scalar engine was the attention
    bottleneck at ~130us/layer; the 352-cycle per-instr overhead dominated)
  - QK/V evacuations moved from scalar (ACTIVATE Identity) to DVE
    tensor_scalar, so the scalar engine runs exp exclusively in attention
  - software pipelining: qk_tile(m+1) GEMMs are emitted between scores(m)
    and AV(m) so the PE has work while exp runs
  - softmax 1/Z row broadcast via gpsimd partition_broadcast instead of a
    DRAM round-trip
  - residual adds fused into PSUM-evacuation scalar_tensor_tensor on DVE
    (the identity-matmul trick is gone; frees ~7us/layer of PE)
  - cross-layer pipeline: next layer's V GEMM is emitted during the tail
    LN/transpose so the PE never drains at layer boundaries (HAM stays warm)
  - optional fp8 DoubleRow FFN (BASS_FFN8=1): both FFN GEMMs at 2x PE rate;
    h2/g activations quantized e4m3 (validated against a numpy quant-sim of
    the full 12-layer error accumulation)
"""

import os
import sys
from contextlib import ExitStack

for _p in ("/opt/trn_rl_repo", "/root/.axon_site/_ro/trn_rl_repo"):
    if os.path.isdir(_p) and _p not in sys.path:
        sys.path.insert(0, _p)

import numpy as np
import ml_dtypes

import concourse.bass as bass
import concourse.tile as tile
from concourse import bacc, mybir
from concourse.bass_utils import run_bass_kernel_spmd

dt = mybir.dt
AF = mybir.ActivationFunctionType
OP = mybir.AluOpType
DR = mybir.MatmulPerfMode.DoubleRow

B, LS = 8, 512
L, D, H, HD, FF = 1024, 1024, 16, 64, 4096
NL = int(os.environ.get("BASS_NL", "12"))
FFN8 = os.environ.get("BASS_FFN8", "0") == "1"
DBG = os.environ.get("BASS_DBG", "")
GPB = os.environ.get("BASS_GPB", "1") == "1"
NT = L // 128    # 8 token tiles
ND = D // 128    # 8 d_model tiles
ND2 = ND // 2    # 4 DoubleRow k-pairs
NF = FF // 128   # 32 ffn tiles
NF2 = NF // 2    # 16 ffn DoubleRow pairs
EPS = 1e-5

SW = 64.0        # attention weight scale
AH = 4.0         # LN-output activation scale (hT, h2T)
AV_ = 4.0        # v scale
AO = 4.0         # o scale (must equal AV_ for the free ones-trick requant)
DQ_H = 1.0 / (SW * AH)          # psum dequant for h-GEMMs: 1/256
SW1 = 64.0       # w1 fp8 scale
SW2 = 64.0       # w2 fp8 scale

f32, f16, f8 = dt.float32, dt.float16, dt.float8e4
F8NP = ml_dtypes.float8_e4m3


def build_nc(nl=NL, has_qb=True, has_ob=True, has_b1=True, has_b2=True):
    nc = bacc.Bacc("TRN2", target_bir_lowering=False, debug=False, num_devices=8)

    x_d = nc.dram_tensor("x0", [L, D], f32, kind="ExternalInput")
    wqk_d = nc.dram_tensor("wqk", [nl, 16, 128, ND2, 2, 128], f8, kind="ExternalInput")
    wqv_d = nc.dram_tensor("wqv", [nl, 128, ND, 1024], f8, kind="ExternalInput")
    bq_d = nc.dram_tensor("bq", [nl, 128, ND], f32, kind="ExternalInput")
    wo_d = nc.dram_tensor("wo", [nl, 128, ND2, 2, 1024], f8, kind="ExternalInput")
    bo_d = nc.dram_tensor("bo", [nl, 128, D], f16, kind="ExternalInput")
    if FFN8:
        w1_d = nc.dram_tensor("w1", [nl, NF, 128, ND2, 2, 128], f8, kind="ExternalInput")
    else:
        w1_d = nc.dram_tensor("w1", [nl, NF, 128, ND, 128], f16, kind="ExternalInput")
    b1_d = nc.dram_tensor("b1", [nl, 128, NF], f32, kind="ExternalInput")
    if FFN8:
        w2_d = nc.dram_tensor("w2", [nl, 2, NF2, 128, 2, 512], f8, kind="ExternalInput")
    else:
        w2_d = nc.dram_tensor("w2", [nl, 2, NF2, 128, 2, 512], f16, kind="ExternalInput")
    b2_d = nc.dram_tensor("b2", [nl, 128, D], f16, kind="ExternalInput")
    fs_d = nc.dram_tensor("flns", [128, D], f32, kind="ExternalInput")
    fb_d = nc.dram_tensor("flnb", [128, D], f32, kind="ExternalInput")
    id_d = nc.dram_tensor("ident", [128, 128], f16, kind="ExternalInput")
    mk_d = nc.dram_tensor("mbias", [128, 128], f16, kind="ExternalInput")
    out_d = nc.dram_tensor("out", [L, D], f32, kind="ExternalOutput")

    GDQ = 1.0 / (SW1 * AH)   # FFN1 psum dequant (h2T carries AH)

    with tile.TileContext(nc) as tc, ExitStack() as ctx:
        def pool(name, bufs, space="SBUF"):
            return ctx.enter_context(tc.tile_pool(name=name, bufs=bufs, space=space))

        cpool = pool("consts", 1)       # ident, mask2, eps tiles
        small = pool("small", 8)        # LN stats
        rzp = pool("rz", 3)             # [1,512] f32 recip of Z
        bcp = pool("bcs", 3)            # [64,512] f32 broadcast recip
        xpool = pool("x", 1)            # residual fp16, 16KB
        f4k = pool("f4k", 2)            # x-load + final-LN consts, f32 4KB
        hpool = pool("h", 6)            # LN output per t-tile, f16 2KB
        htpool = pool("hT", 2)          # transposed LN output fp8, 8KB
        big = pool("big", 4)            # qT,kT (f16) / v,oT + 4 gT
        apool = pool("aT", 3)           # exp(scores) fp8 [128,2,8,512], 8KB
        wsp = pool("wstream", 3)        # streamed QK/W1 weight tiles
        w16 = pool("w16", 1)            # V-part / out-proj weights fp8 8KB
        w2p = pool("w2s", 6)            # streamed GEMM2 weight tiles
        bpool = pool("bias", 1)         # per-layer bias tiles
        dpool = pool("dscr", 4, space="DRAM")  # 1/Z DRAM bounce (GPB=0 path)
        psC = pool("psC", 2, space="PSUM")   # scores: 2x [128,2,512]
        psB = pool("psB", 2, space="PSUM")   # everything else: 2x 2 banks

        ident = cpool.tile([128, 128], f16)
        nc.sync.dma_start(ident[:], id_d[:])
        mbias = cpool.tile([128, 128], f16)
        nc.sync.dma_start(mbias[:], mk_d[:])
        epsa = cpool.tile([128, 1], f32)
        nc.gpsimd.memset(epsa[:], EPS / (AH * AH))
        epst = cpool.tile([128, 1], f32)
        nc.gpsimd.memset(epst[:], EPS)

        # ---- initial x = (sensor|traj + pos), cast to fp16 ----
        x = xpool.tile([128, NT, D], f16)
        for i in range(NT):
            xt = f4k.tile([128, D], f32, tag="f4k")
            nc.sync.dma_start(xt[:], x_d[i * 128:(i + 1) * 128, :])
            nc.vector.tensor_copy(x[:, i, :], xt[:])

        def ln_group(tiles, hbuf, epsv, inva, tagsuf=""):
            """LN stats+normalize for a group of t-tiles; one batched sqrt."""
            ntl = len(tiles)
            mvb = small.tile([128, ntl, 2], f32, tag="mvb", name=f"mvb{tagsuf}")
            for t, i in enumerate(tiles):
                st = small.tile([128, 12], f32, tag="stats")
                nc.vector.bn_stats(st[:, 0:6], x[:, i, 0:512])
                nc.vector.bn_stats(st[:, 6:12], x[:, i, 512:1024])
                nc.vector.bn_aggr(mvb[:, t, :], st[:])
            stdb = small.tile([128, ntl], f32, tag="stdb", name=f"stdb{tagsuf}")
            nc.scalar.activation(stdb[:], mvb[:, :, 1], AF.Sqrt,
                                 bias=epsv[:], scale=inva)
            rstdb = small.tile([128, ntl], f32, tag="rstdb", name=f"rstdb{tagsuf}")
            nc.vector.reciprocal_approx_fast(rstdb[:], stdb[:])
            nmrb = small.tile([128, ntl], f32, tag="nmrb", name=f"nmrb{tagsuf}")
            nc.vector.tensor_mul(nmrb[:], mvb[:, :, 0], rstdb[:])
            for t, i in enumerate(tiles):
                hi = hpool.tile([128, D], f16, tag="h", name=f"h{tagsuf}_{i}")
                # h = x*rstd - mean*rstd
                nc.vector.tensor_scalar(hi[:], x[:, i, :], rstdb[:, t:t + 1],
                                        nmrb[:, t:t + 1], OP.mult, OP.subtract)
                hbuf[i] = hi

        def lnT_pair(hbuf, i, dstT):
            """PE-transpose tiles i, i+1 into dstT columns; one DVE evac."""
            tp = psB.tile([128, 2, ND, 128], f16, tag="wk", name=f"tp_{i}")
            for t in range(2):
                hi = hbuf[i + t]
                for j in range(ND):
                    nc.tensor.transpose(
                        tp[:, t, j, :], hi[:, j * 128:(j + 1) * 128], ident[:]
                    )
            # dst [128, ND, 256] <- src reordered (j, t)
            src = tp[:].rearrange("p t j c -> p j t c")
            nc.vector.tensor_copy(dstT[:, :, i * 128:(i + 2) * 128].rearrange(
                "p j (t c) -> p j t c", c=128), src)

        hT_next = None
        wqv = None

        def v_gemm(tiles, v, wqv):
            for i in tiles:
                vv = v[:, i, :].rearrange("p (h e) -> p h e", e=65)
                nc.gpsimd.memset(vv[:, :, 64:65], AV_ / AO)
                pt = psB.tile([128, 2, 512], f32, tag="wk", name=f"vps_{i}")
                for j2 in range(ND2):
                    for c in range(2):
                        nc.tensor.matmul(
                            pt[:, c, :],
                            hT[:, 2 * j2:2 * j2 + 2, i * 128:(i + 1) * 128],
                            wqv[:, 2 * j2:2 * j2 + 2, c * 512:(c + 1) * 512],
                            start=(j2 == 0),
                            stop=(j2 == ND2 - 1),
                            perf_mode=DR,
                        )
                nc.vector.tensor_scalar_mul(
                    vv[:, :, 0:64].rearrange("p (c h) e -> p c h e", c=2),
                    pt[:].rearrange("p c (h e) -> p c h e", e=64),
                    AV_ * DQ_H,
                )

        for l in range(nl):
            # ================= attention =================
            if l == 0:
                hT = htpool.tile([128, ND, L], f8, tag="hT")
                hbuf = {}
                wqv = w16.tile([128, ND, 1024], f8, tag="w16", name="wqv0")
                nc.sync.dma_start(wqv[:], wqv_d[0])
                v = big.tile([128, NT, 16 * 65], f8, tag="big", name="v0")
                for pr in range(4):
                    i0 = 2 * pr
                    ln_group((i0, i0 + 1), hbuf, epsa, 1.0 / (AH * AH),
                             f"i{i0}")
                    lnT_pair(hbuf, i0, hT)
                    v_gemm((i0, i0 + 1), v, wqv)
            else:
                hT = hT_next
                v = v_pending

            if has_qb:
                bq_sb = bpool.tile([128, ND], f32, tag="bq")
                nc.sync.dma_start(bq_sb[:], bq_d[l])

            if l != 0:
                v_gemm(range(4, 8), v, wqv)

            # wo prefetch (w16 slot frees once wqv is released after V GEMM)
            wo_sb = w16.tile([128, ND2, 2, 1024], f8, tag="w16", name=f"wo{l}")
            nc.sync.dma_start(wo_sb[:], wo_d[l])
            if has_ob:
                bo_sb = bpool.tile([128, D], f16, tag="bo")
                nc.sync.dma_start(bo_sb[:], bo_d[l])

            qT = big.tile([128, ND, 1040], f16, tag="big", name=f"qT{l}")
            kT = big.tile([128, ND, 1040], f16, tag="big", name=f"kT{l}")
            oT = big.tile([128, ND, 1040], f8, tag="big", name=f"oT{l}")

            def qk_tile(m):
                wt = wsp.tile([128, ND2, 2, 128], f8, tag="ws", name=f"wt_{l}_{m}")
                nc.sync.dma_start(wt[:], wqk_d[l, m])
                dst = qT if m < ND else kT
                pt = psB.tile([128, 2, 512], f32, tag="wk", name=f"qkps_{l}_{m}")
                for j2 in range(ND2):
                    for c in range(2):
                        nc.tensor.matmul(
                            pt[:, c, :],
                            wt[:, j2, :, :],
                            hT[:, 2 * j2:2 * j2 + 2, c * 512:(c + 1) * 512],
                            start=(j2 == 0),
                            stop=(j2 == ND2 - 1),
                            perf_mode=DR,
                        )
                if m < ND and has_qb:  # Q bias (per-partition)
                    nc.vector.tensor_scalar(
                        dst[:, m, 0:1024], pt[:].rearrange("p c q -> p (c q)"),
                        DQ_H, bq_sb[:, m:m + 1], OP.mult, OP.add)
                else:
                    nc.vector.tensor_scalar_mul(
                        dst[:, m % ND, 0:1024],
                        pt[:].rearrange("p c q -> p (c q)"), DQ_H)

            def score_j(jo, c, j, aT):
                """Score MMs + merged exp for one key tile j (both heads)."""
                w0 = max(0, (j - 4) * 128) if c == 1 else 0
                diag = (c == 1 and j >= 4)
                sc = psC.tile([128, 2, 512], f32, tag="sc",
                              name=f"sc_{l}_{jo}_{c}_{j}")
                for pi, po in enumerate((0, 64)):
                    nc.tensor.matmul(
                        sc[:, pi, w0:512],
                        kT[po:po + 64, jo, j * 128:(j + 1) * 128],
                        qT[po:po + 64, jo, c * 512 + w0:(c + 1) * 512],
                        start=True, stop=not diag,
                    )
                if diag:
                    # additive -3e4 on the strict upper triangle of the
                    # diagonal block, via a const matmul into the same PSUM
                    for pi in (0, 1):
                        nc.tensor.matmul(
                            sc[:, pi, w0:w0 + 128], mbias[:], ident[:],
                            start=False, stop=True,
                        )
                nc.scalar.activation(aT[:, :, j, w0:512], sc[:, :, w0:512],
                                     AF.Exp, scale=0.125)
                # zero the strips the AV pair reads but exp never writes
                if c == 1 and j == 5:
                    nc.gpsimd.memset(aT[:, :, 5, 0:128], 0.0)
                if c == 1 and j == 7:
                    nc.gpsimd.memset(aT[:, :, 7, 256:384], 0.0)

            def av_pairs(jo, c, aT, us):
                """AV DoubleRow accumulation for pair indices us (list)."""
                nkt = 4 if c == 0 else 8
                np2 = nkt // 2
                for u in us:
                    j = 2 * u
                    w0 = max(0, (j - 4) * 128)
                    for pi in (0, 1):
                        hh = 2 * jo + pi
                        nc.tensor.matmul(
                            opsl[0:65, pi, w0:512],
                            v[:, j:j + 2, 65 * hh:65 * hh + 65],
                            aT[:, pi, j:j + 2, w0:512],
                            start=(u == 0),
                            stop=(u == np2 - 1),
                            perf_mode=DR,
                        )

            def z_chain(jo, c):
                for pi, po in enumerate((0, 64)):
                    rz = rzp.tile([1, 512], f32, tag="rz",
                                  name=f"rz_{l}_{jo}_{c}_{pi}")
                    bcs = bcp.tile([64, 512], f32, tag="bcs",
                                   name=f"bc_{l}_{jo}_{c}_{pi}")
                    if GPB:
                        nc.vector.tensor_copy(rz[:], opsl[64:65, pi, :])
                        nc.vector.reciprocal_approx_fast(rz[:], rz[:])
                        nc.gpsimd.partition_broadcast(bcs[:], rz[:], channels=64)
                    else:
                        # v2-style: bounce Z through DRAM to broadcast, then recip
                        nc.vector.tensor_copy(rz[:], opsl[64:65, pi, :])
                        rzd = dpool.tile([512], f32, tag="rzd",
                                         name=f"rzd_{l}_{jo}_{c}_{pi}")
                        nc.sync.dma_start(rzd[:].unsqueeze(0), rz[:])
                        nc.sync.dma_start(
                            bcs[:], rzd[:].unsqueeze(0).broadcast_to((64, 512)))
                        nc.vector.reciprocal_approx_fast(bcs[:], bcs[:])
                    nc.vector.tensor_mul(
                        oT[po:po + 64, jo, c * 512:(c + 1) * 512],
                        opsl[0:64, pi, :], bcs[:]
                    )

            def dbg_dump_t(src_ap, j, cols=1024):
                xo = f4k.tile([128, D], f32, tag="xo", name=f"dbg_{l}_{j}")
                nc.vector.tensor_copy(xo[:, 0:cols], src_ap)
                nc.sync.dma_start(out_d[j * 128:(j + 1) * 128, 0:cols], xo[:, 0:cols])

            if DBG == "hT" and l == 0:
                for j in range(ND):
                    dbg_dump_t(hT[:, j, 0:1024], j)
                break
            if DBG == "v" and l == 0:
                for i in range(NT):
                    dbg_dump_t(v[:, i, 0:1024], i)
                break

            qk_tile(0)
            qk_tile(ND)
            if DBG in ("qT", "kT") and l == 0:
                for m in range(1, ND):
                    qk_tile(m)
                    qk_tile(ND + m)
                src = qT if DBG == "qT" else kT
                for j in range(ND):
                    dbg_dump_t(src[:, j, 0:1024], j)
                break
            for jo in range(ND):
                aT0 = apool.tile([128, 2, 8, 512], f8, tag="aT",
                                 name=f"aT_{l}_{jo}_0")
                aT1 = apool.tile([128, 2, 8, 512], f8, tag="aT",
                                 name=f"aT_{l}_{jo}_1")
                score_j(jo, 0, 0, aT0)
                score_j(jo, 0, 1, aT0)
                if jo < ND - 1:
                    qk_tile(jo + 1)
                score_j(jo, 0, 2, aT0)
                score_j(jo, 0, 3, aT0)
                opsl = psB.tile([128, 2, 512], f32, tag="wk",
                                name=f"op_{l}_{jo}_0")
                av_pairs(jo, 0, aT0, (0,))
                score_j(jo, 1, 0, aT1)
                score_j(jo, 1, 1, aT1)
                av_pairs(jo, 0, aT0, (1,))
                z_chain(jo, 0)
                if jo < ND - 1:
                    qk_tile(ND + jo + 1)
                score_j(jo, 1, 2, aT1)
                score_j(jo, 1, 3, aT1)
                opsl = psB.tile([128, 2, 512], f32, tag="wk",
                                name=f"op_{l}_{jo}_1")
                av_pairs(jo, 1, aT1, (0,))
                score_j(jo, 1, 4, aT1)
                score_j(jo, 1, 5, aT1)
                av_pairs(jo, 1, aT1, (1,))
                score_j(jo, 1, 6, aT1)
                score_j(jo, 1, 7, aT1)
                av_pairs(jo, 1, aT1, (2,))
                av_pairs(jo, 1, aT1, (3,))
                z_chain(jo, 1)
                if DBG == "aT" and l == 0 and jo == 0:
                    for j in range(8):
                        dbg_dump_t(aT0[:, 0, j, :], j, cols=512)
                    for j in range(8):
                        xo = f4k.tile([128, D], f32, tag="xo", name=f"dbgb_{j}")
                        nc.vector.tensor_copy(xo[:, 0:512], aT1[:, 0, j, :])
                        nc.sync.dma_start(out_d[j * 128:(j + 1) * 128, 512:1024],
                                          xo[:, 0:512])
                    break

            if DBG == "aT" and l == 0:
                break
            if DBG == "oT" and l == 0:
                for j in range(ND):
                    dbg_dump_t(oT[:, j, 0:1024], j)
                break

            # out-proj + residual fused in PSUM-evac, then LN2 per 4-tile
            # group so the DVE work overlaps the PE
            h2T = htpool.tile([128, ND, L], f8 if FFN8 else f16, tag="hT",
                              name=f"h2T{l}")
            h2buf = {}

            def out_proj_mm(i):
                yp = psB.tile([128, 2, 512], f32, tag="wk", name=f"ops_{i}")
                for j2 in range(ND2):
                    for c in range(2):
                        nc.tensor.matmul(
                            yp[:, c, :],
                            oT[:, 2 * j2:2 * j2 + 2, i * 128:(i + 1) * 128],
                            wo_sb[:, j2, :, c * 512:(c + 1) * 512],
                            start=(j2 == 0),
                            stop=(j2 == ND2 - 1),
                            perf_mode=DR,
                        )
                return yp

            def out_proj_ev(i, yp):
                # x += psum/(AO*SW)  (one fused DVE op)
                nc.vector.scalar_tensor_tensor(
                    x[:, i, :], yp[:].rearrange("p c q -> p (c q)"),
                    1.0 / (AO * SW), x[:, i, :], OP.mult, OP.add)
                if has_ob:
                    nc.vector.tensor_add(x[:, i, :], x[:, i, :], bo_sb[:])

            def out_proj(tiles, ev=True):
                yps = []
                for i in tiles:
                    yp = out_proj_mm(i)
                    if ev:
                        out_proj_ev(i, yp)
                    else:
                        yps.append((i, yp))
                return yps

            ep2 = epsa if FFN8 else epst
            iv2 = 1.0 / (AH * AH) if FFN8 else 1.0
            out_proj(range(0, 4))
            ln_group(range(0, 4), h2buf, ep2, iv2, f"a{l}")
            out_proj((4, 5))
            lnT_pair(h2buf, 0, h2T)
            out_proj((6, 7))
            lnT_pair(h2buf, 2, h2T)
            ln_group(range(4, 8), h2buf, ep2, iv2, f"b{l}")

            if DBG == "xattn" and l == 0:
                lnT_pair(h2buf, 4, h2T)
                lnT_pair(h2buf, 6, h2T)
                for i in range(NT):
                    dbg_dump_t(x[:, i, :], i)
                break

            # ================= FFN =================
            if has_b1:
                b1_sb = bpool.tile([128, NF], f32, tag="b1")
                nc.sync.dma_start(b1_sb[:], b1_d[l])
            if has_b2:
                b2_sb = bpool.tile([128, D], f16, tag="b2")
                nc.sync.dma_start(b2_sb[:], b2_d[l])

            # wqv for next layer (slot frees after this layer's out_proj)
            if l != nl - 1:
                wqv = w16.tile([128, ND, 1024], f8, tag="w16", name=f"wqv{l+1}")
                nc.sync.dma_start(wqv[:], wqv_d[l + 1])

            # GEMM1 (+ gelu) -> g^T [ff(P), t] quarters; c-halves split so
            # the c=0 GEMMs start as soon as token tiles 0-3 are transposed
            gq = []
            for q in range(4):
                g = big.tile([128, 8, 1040], f8 if FFN8 else f16, tag="big",
                             name=f"gT_{l}_{q}")
                gq.append(g)

            def ffn1_half(c):
                for f2 in range(NF // 2):
                    gp = psB.tile([128, 2, 512], f32, tag="wk",
                                  name=f"g1_{c}_{f2}")
                    for ff in range(2):
                        f = 2 * f2 + ff
                        if FFN8:
                            w1t = wsp.tile([128, ND2, 2, 128], f8, tag="ws",
                                           name=f"w1_{l}_{c}_{f}")
                            nc.sync.dma_start(w1t[:], w1_d[l, f])
                            for j2 in range(ND2):
                                nc.tensor.matmul(
                                    gp[:, ff, :],
                                    w1t[:, j2, :, :],
                                    h2T[:, 2 * j2:2 * j2 + 2,
                                        c * 512:(c + 1) * 512],
                                    start=(j2 == 0),
                                    stop=(j2 == ND2 - 1),
                                    perf_mode=DR,
                                )
                        else:
                            w1t = wsp.tile([128, ND, 128], f16, tag="ws",
                                           name=f"w1_{l}_{c}_{f}")
                            nc.sync.dma_start(w1t[:], w1_d[l, f])
                            for j in range(ND):
                                nc.tensor.matmul(
                                    gp[:, ff, :],
                                    w1t[:, j, :],
                                    h2T[:, j, c * 512:(c + 1) * 512],
                                    start=(j == 0),
                                    stop=(j == ND - 1),
                                )
                    gsc = GDQ if FFN8 else 1.0
                    q, fo = f2 // 4, 2 * (f2 % 4)
                    dst = gq[q][:, fo:fo + 2, c * 512:(c + 1) * 512]
                    if has_b1:
                        for ff in range(2):
                            nc.scalar.activation(
                                gq[q][:, fo + ff, c * 512:(c + 1) * 512],
                                gp[:, ff, :], AF.Gelu,
                                bias=b1_sb[:, 2 * f2 + ff:2 * f2 + ff + 1],
                                scale=gsc)
                    else:
                        nc.scalar.activation(dst, gp[:], AF.Gelu, scale=gsc)

            ffn1_half(0)
            lnT_pair(h2buf, 4, h2T)
            lnT_pair(h2buf, 6, h2T)
            ffn1_half(1)

            # GEMM2: acc pairs (2 t-tiles per 2-bank slot), w2 streamed
            last = nl - 1
            hT_next = None if l == last else htpool.tile(
                [128, ND, L], f8, tag="hT", name=f"hTn_{l}")
            hnbuf = {}

            def ffn2(tg):
                for c in range(2):
                    cs = slice(c * 512, (c + 1) * 512)
                    ys = [psB.tile([128, 2, 512], f32, tag="wk",
                                   name=f"psy_{l}_{c}_{tg[0]}_{k}")
                          for k in range((len(tg) + 1) // 2)]
                    if FFN8:
                        for f2 in range(NF2):
                            w2t = w2p.tile([128, 2, 512], f8, tag="w2s")
                            nc.sync.dma_start(w2t[:], w2_d[l, c, f2])
                            q, fo = f2 // 4, 2 * (f2 % 4)
                            for k, i in enumerate(tg):
                                nc.tensor.matmul(
                                    ys[k // 2][:, k % 2, :],
                                    gq[q][:, fo:fo + 2, i * 128:(i + 1) * 128],
                                    w2t[:],
                                    start=(f2 == 0),
                                    stop=(f2 == NF2 - 1),
                                    perf_mode=DR,
                                )
                        dq = 1.0 / SW2
                    else:
                        for f2 in range(NF2):
                            w2t = w2p.tile([128, 2, 512], f16, tag="w2s")
                            nc.sync.dma_start(w2t[:], w2_d[l, c, f2])
                            for ff in range(2):
                                f = 2 * f2 + ff
                                for k, i in enumerate(tg):
                                    nc.tensor.matmul(
                                        ys[k // 2][:, k % 2, :],
                                        gq[f // 8][:, f % 8,
                                           i * 128:(i + 1) * 128],
                                        w2t[:, ff, :],
                                        start=(f == 0),
                                        stop=(f == NF - 1),
                                    )
                        dq = 1.0
                    for k, i in enumerate(tg):
                        nc.vector.scalar_tensor_tensor(
                            x[:, i, cs], ys[k // 2][:, k % 2, :], dq,
                            x[:, i, cs], OP.mult, OP.add)
                        if has_b2:
                            nc.vector.tensor_add(x[:, i, cs], x[:, i, cs],
                                                 b2_sb[:, cs])

            epn = epsa
            ivn = 1.0 / (AH * AH)
            if l != last:
                ffn2((0, 1, 2, 3))
                ln_group(range(0, 4), hnbuf, epn, ivn, f"n0{l}")
                ffn2((4, 5))
                ln_group((4, 5), hnbuf, epn, ivn, f"n4{l}")
                lnT_pair(hnbuf, 0, hT_next)
                lnT_pair(hnbuf, 2, hT_next)
                ffn2((6, 7))
                ln_group((6, 7), hnbuf, epn, ivn, f"n6{l}")
                v_pending = big.tile([128, NT, 16 * 65], f8, tag="big",
                                     name=f"v{l+1}")
                # early V GEMM for next layer (tiles 0-3) while DVE does LN
                hT = hT_next
                v_gemm(range(0, 4), v_pending, wqv)
                lnT_pair(hnbuf, 4, hT_next)
                lnT_pair(hnbuf, 6, hT_next)
            else:
                # final layer: fuse the final LN + store into the FFN2 tail
                flns = f4k.tile([128, D], f32, tag="f4k")
                nc.sync.dma_start(flns[:], fs_d[:])
                flnb = f4k.tile([128, D], f32, tag="f4k")
                nc.sync.dma_start(flnb[:], fb_d[:])

                def fln_emit(tiles, suf):
                    fbuf = {}
                    ln_group(tiles, fbuf, epst, 1.0, suf)
                    for i in tiles:
                        xo = f4k.tile([128, D], f32, tag="xo", name=f"xo_{i}")
                        nc.vector.tensor_mul(xo[:], fbuf[i][:], flns[:])
                        nc.vector.tensor_add(xo[:], xo[:], flnb[:])
                        nc.sync.dma_start(out_d[i * 128:(i + 1) * 128, :], xo[:])

                ffn2((0, 1, 2, 3))
                fln_emit(range(0, 4), "f0")
                ffn2((4, 5))
                fln_emit((4, 5), "f4")
                ffn2((6, 7))
                fln_emit((6, 7), "f6")

    nc.compile()
    return nc


def _host_prep(sensor_tokens, traj_tokens, pos_embed, ln1_s, ln1_b,
               qkv_w, qkv_b, out_w, out_b, ln2_s, ln2_b,
               w1, b1, w2, b2, fln_s, fln_b, nl=NL):
    """Fold LN affine params into weights; retile + fp8-cast with scaling."""
    fp = np.float32
    x_all = np.concatenate([sensor_tokens, traj_tokens], axis=1).astype(fp)
    x_all = x_all + pos_embed[:L][None].astype(fp)

    wqk8 = np.empty((nl, 16, 128, ND2, 2, 128), F8NP)
    wqv8 = np.empty((nl, 128, ND, 1024), F8NP)
    bqh = np.empty((nl, 128, ND), fp)
    wo8 = np.empty((nl, 128, ND2, 2, 1024), F8NP)
    boh = np.empty((nl, 128, D), np.float16)
    if FFN8:
        w1T = np.empty((nl, NF, 128, ND2, 2, 128), F8NP)
        w2T = np.empty((nl, 2, NF2, 128, 2, 512), F8NP)
    else:
        w1T = np.empty((nl, NF, 128, ND, 128), np.float16)
        w2T = np.empty((nl, 2, NF2, 128, 2, 512), np.float16)
    b1h = np.empty((nl, 128, NF), fp)
    b2h = np.empty((nl, 128, D), np.float16)

    for i in range(nl):
        Wq = qkv_w[i].astype(fp)                                  # [3D, D]
        bfull = qkv_b[i].astype(fp) + Wq @ ln1_b[i].astype(fp)    # [3D]
        Wq = Wq * ln1_s[i].astype(fp)[None, :]
        WqT = (Wq.T * SW).astype(F8NP)                            # [D, 3D]
        qk = WqT[:, :2 * D].reshape(ND2, 2, 128, 16, 128)
        wqk8[i] = qk.transpose(3, 2, 0, 1, 4)
        wqv8[i] = WqT[:, 2 * D:].reshape(ND, 128, 1024).transpose(1, 0, 2)
        bqh[i] = bfull[:D].reshape(ND, 128).T
        bv = bfull[2 * D:]
        Wo = out_w[i].astype(fp)                                  # [D, D]
        bo = out_b[i].astype(fp) + Wo @ bv
        wo8[i] = (Wo.T * SW).astype(F8NP).reshape(ND2, 2, 128, 1024).transpose(2, 0, 1, 3)
        boh[i] = np.broadcast_to(bo.astype(np.float16), (128, D))
        W1 = w1[i].astype(fp)                                     # [FF, D]
        b1f = b1[i].astype(fp) + W1 @ ln2_b[i].astype(fp)
        W1 = W1 * ln2_s[i].astype(fp)[None, :]
        if FFN8:
            W1t = (W1.T * SW1).astype(F8NP)                       # [D, FF]
            # [d, ff] -> (f, p(ffchunk? no: p=d%128), j2, jj, col)
            w1T[i] = W1t.reshape(ND2, 2, 128, NF, 128).transpose(3, 2, 0, 1, 4)
            W2t = (w2[i].astype(fp).T * SW2).astype(F8NP)         # [FF, D]
            w2T[i] = W2t.reshape(NF2, 2, 128, 2, 512).transpose(3, 0, 2, 1, 4)
        else:
            W1t = W1.T.astype(np.float16)                         # [D, FF]
            w1T[i] = W1t.reshape(ND, 128, NF, 128).transpose(2, 1, 0, 3)
            W2t = w2[i].astype(fp).T.astype(np.float16)           # [FF, D]
            w2T[i] = W2t.reshape(NF2, 2, 128, 2, 512).transpose(3, 0, 2, 1, 4)
        b1h[i] = b1f.reshape(NF, 128).T
        b2h[i] = np.broadcast_to(b2[i].astype(np.float16), (128, D))

    common = dict(
        wqk=wqk8, wqv=wqv8, bq=bqh, wo=wo8, bo=boh,
        w1=w1T, b1=b1h, w2=w2T, b2=b2h,
        flns=np.broadcast_to(fln_s.astype(fp), (128, D)).copy(),
        flnb=np.broadcast_to(fln_b.astype(fp), (128, D)).copy(),
        ident=np.eye(128, dtype=np.float16),
        mbias=np.triu(np.full((128, 128), -30000.0, np.float16), 1),
    )
    in_maps = [dict(common, x0=np.ascontiguousarray(x_all[c])) for c in range(B)]
    return in_maps


_NC = {}
LAST_RESULT = None


def kernel(**inputs):
    global LAST_RESULT
    in_maps = _host_prep(**inputs)
    m0 = in_maps[0]
    flags = (bool(np.any(m0["bq"])), bool(np.any(m0["bo"])),
             bool(np.any(m0["b1"])), bool(np.any(m0["b2"])))
    if flags not in _NC:
        _NC[flags] = build_nc(NL, *flags)
    res = run_bass_kernel_spmd(_NC[flags], in_maps, core_ids=list(range(B)))
    LAST_RESULT = res
    return np.stack([res.results[c]["out"] for c in range(B)]).astype(np.float32)


# revision 20
# speedup vs baseline: 1.0734x; 1.0734x over previous
"""Trainium2 Bass kernel for a 12-layer prefix-causal transformer.

Sharding: data-parallel over batch B=8 across 8 NeuronCores (1 sequence per
core, weights replicated, no collectives).

v3 (from v2's fp8 DoubleRow attention baseline), trace-driven restructure:
  - scores exp merged into multi-bank PSUM groups: 6 big ACTIVATEs per
    head-pair instead of 24 small ones (scalar engine was the attention
    bottleneck at ~130us/layer; the 352-cycle per-instr overhead dominated)
  - QK/V evacuations moved from scalar (ACTIVATE Identity) to DVE
    tensor_scalar, so the scalar engine runs exp exclusively in attention
  - software pipelining: qk_tile(m+1) GEMMs are emitted between scores(m)
    and AV(m) so the PE has work while exp runs
  - softmax 1/Z row broadcast via gpsimd partition_broadcast instead of a
    DRAM round-trip
  - residual adds fused into PSUM-evacuation scalar_tensor_tensor on DVE
    (the identity-matmul trick is gone; frees ~7us/layer of PE)
  - cross-layer pipeline: next layer's V GEMM is emitted during the tail
    LN/transpose so the PE never drains at layer boundaries (HAM stays warm)
  - optional fp8 DoubleRow FFN (BASS_FFN8=1): both FFN GEMMs at 2x PE rate;
    h2/g activations quantized e4m3 (validated against a numpy quant-sim of
    the full 12-layer error accumulation)
"""

import os
import sys
from contextlib import ExitStack

for _p in ("/opt/trn_rl_repo", "/root/.axon_site/_ro/trn_rl_repo"):
    if os.path.isdir(_p) and _p not in sys.path:
        sys.path.insert(0, _p)

import numpy as np
import ml_dtypes

import concourse.bass as bass
import concourse.tile as tile
from concourse import bacc, mybir
from concourse.bass_utils import run_bass_kernel_spmd

dt = mybir.dt
AF = mybir.ActivationFunctionType
OP = mybir.AluOpType
DR = mybir.MatmulPerfMode.DoubleRow

B, LS = 8, 512
L, D, H, HD, FF = 1024, 1024, 16, 64, 4096
NL = int(os.environ.get("BASS_NL", "12"))
FFN8 = os.environ.get("BASS_FFN8", "0") == "1"
DBG = os.environ.get("BASS_DBG", "")
GPB = os.environ.get("BASS_GPB", "1") == "1"
NT = L // 128    # 8 token tiles
ND = D // 128    # 8 d_model tiles
ND2 = ND // 2    # 4 DoubleRow k-pairs
NF = FF // 128   # 32 ffn tiles
NF2 = NF // 2    # 16 ffn DoubleRow pairs
EPS = 1e-5

SW = 64.0        # attention weight scale
AH = 4.0         # LN-output activation scale (hT, h2T)
AV_ = 4.0        # v scale
AO = 4.0         # o scale (must equal AV_ for the free ones-trick requant)
DQ_H = 1.0 / (SW * AH)          # psum dequant for h-GEMMs: 1/256
SW1 = 64.0       # w1 fp8 scale
SW2 = 64.0       # w2 fp8 scale

f32, f16, f8 = dt.float32, dt.float16, dt.float8e4
F8NP = ml_dtypes.float8_e4m3


def build_nc(nl=NL, has_qb=True, has_ob=True, has_b1=True, has_b2=True):
    nc = bacc.Bacc("TRN2", target_bir_lowering=False, debug=False, num_devices=8)

    x_d = nc.dram_tensor("x0", [L, D], f32, kind="ExternalInput")
    wqk_d = nc.dram_tensor("wqk", [nl, 16, 128, ND2, 2, 128], f8, kind="ExternalInput")
    wqv_d = nc.dram_tensor("wqv", [nl, 128, ND, 1024], f8, kind="ExternalInput")
    bq_d = nc.dram_tensor("bq", [nl, 128, ND], f32, kind="ExternalInput")
    wo_d = nc.dram_tensor("wo", [nl, 128, ND2, 2, 1024], f8, kind="ExternalInput")
    bo_d = nc.dram_tensor("bo", [nl, 128, D], f16, kind="ExternalInput")
    if FFN8:
        w1_d = nc.dram_tensor("w1", [nl, NF, 128, ND2, 2, 128], f8, kind="ExternalInput")
    else:
        w1_d = nc.dram_tensor("w1", [nl, NF, 128, ND, 128], f16, kind="ExternalInput")
    b1_d = nc.dram_tensor("b1", [nl, 128, NF], f32, kind="ExternalInput")
    if FFN8:
        w2_d = nc.dram_tensor("w2", [nl, 2, NF2, 128, 2, 512], f8, kind="ExternalInput")
    else:
        w2_d = nc.dram_tensor("w2", [nl, 2, NF2, 128, 2, 512], f16, kind="ExternalInput")
    b2_d = nc.dram_tensor("b2", [nl, 128, D], f16, kind="ExternalInput")
    fs_d = nc.dram_tensor("flns", [128, D], f32, kind="ExternalInput")
    fb_d = nc.dram_tensor("flnb", [128, D], f32, kind="ExternalInput")
    id_d = nc.dram_tensor("ident", [128, 128], f16, kind="ExternalInput")
    mk_d = nc.dram_tensor("mbias", [128, 128], f16, kind="ExternalInput")
    out_d = nc.dram_tensor("out", [L, D], f32, kind="ExternalOutput")

    GDQ = 1.0 / (SW1 * AH)   # FFN1 psum dequant (h2T carries AH)

    with tile.TileContext(nc) as tc, ExitStack() as ctx:
        def pool(name, bufs, space="SBUF"):
            return ctx.enter_context(tc.tile_pool(name=name, bufs=bufs, space=space))

        cpool = pool("consts", 1)       # ident, mask2, eps tiles
        small = pool("small", 8)        # LN stats
        rzp = pool("rz", 3)             # [1,512] f32 recip of Z
        bcp = pool("bcs", 3)            # [64,512] f32 broadcast recip
        xpool = pool("x", 1)            # residual fp16, 16KB
        f4k = pool("f4k", 2)            # x-load + final-LN consts, f32 4KB
        hpool = pool("h", 6)            # LN output per t-tile, f16 2KB
        htpool = pool("hT", 2)          # transposed LN output fp8, 8KB
        big = pool("big", 4)            # qT,kT (f16) / v,oT + 4 gT
        apool = pool("aT", 3)           # exp(scores) fp8 [128,2,8,512], 8KB
        wsp = pool("wstream", 3)        # streamed QK/W1 weight tiles
        w16 = pool("w16", 1)            # V-part / out-proj weights fp8 8KB
        w2p = pool("w2s", 6)            # streamed GEMM2 weight tiles
        bpool = pool("bias", 1)         # per-layer bias tiles
        dpool = pool("dscr", 4, space="DRAM")  # 1/Z DRAM bounce (GPB=0 path)
        psC = pool("psC", 2, space="PSUM")   # scores: 2x [128,2,512]
        psB = pool("psB", 2, space="PSUM")   # everything else: 2x 2 banks

        ident = cpool.tile([128, 128], f16)
        nc.sync.dma_start(ident[:], id_d[:])
        mbias = cpool.tile([128, 128], f16)
        nc.sync.dma_start(mbias[:], mk_d[:])
        epsa = cpool.tile([128, 1], f32)
        nc.gpsimd.memset(epsa[:], EPS / (AH * AH))
        epst = cpool.tile([128, 1], f32)
        nc.gpsimd.memset(epst[:], EPS)

        # ---- initial x = (sensor|traj + pos), cast to fp16 ----
        x = xpool.tile([128, NT, D], f16)
        for i in range(NT):
            xt = f4k.tile([128, D], f32, tag="f4k")
            nc.sync.dma_start(xt[:], x_d[i * 128:(i + 1) * 128, :])
            nc.vector.tensor_copy(x[:, i, :], xt[:])

        def ln_group(tiles, hbuf, epsv, inva, tagsuf=""):
            """LN stats+normalize for a group of t-tiles; one batched sqrt."""
            ntl = len(tiles)
            mvb = small.tile([128, ntl, 2], f32, tag="mvb", name=f"mvb{tagsuf}")
            for t, i in enumerate(tiles):
                st = small.tile([128, 12], f32, tag="stats")
                nc.vector.bn_stats(st[:, 0:6], x[:, i, 0:512])
                nc.vector.bn_stats(st[:, 6:12], x[:, i, 512:1024])
                nc.vector.bn_aggr(mvb[:, t, :], st[:])
            stdb = small.tile([128, ntl], f32, tag="stdb", name=f"stdb{tagsuf}")
            nc.scalar.activation(stdb[:], mvb[:, :, 1], AF.Sqrt,
                                 bias=epsv[:], scale=inva)
            rstdb = small.tile([128, ntl], f32, tag="rstdb", name=f"rstdb{tagsuf}")
            nc.vector.reciprocal_approx_fast(rstdb[:], stdb[:])
            nmrb = small.tile([128, ntl], f32, tag="nmrb", name=f"nmrb{tagsuf}")
            nc.vector.tensor_mul(nmrb[:], mvb[:, :, 0], rstdb[:])
            for t, i in enumerate(tiles):
                hi = hpool.tile([128, D], f16, tag="h", name=f"h{tagsuf}_{i}")
                # h = x*rstd - mean*rstd
                nc.vector.tensor_scalar(hi[:], x[:, i, :], rstdb[:, t:t + 1],
                                        nmrb[:, t:t + 1], OP.mult, OP.subtract)
                hbuf[i] = hi

        def lnT_pair(hbuf, i, dstT):
            """PE-transpose tiles i, i+1 into dstT columns; one DVE evac."""
            tp = psB.tile([128, 2, ND, 128], f16, tag="wk", name=f"tp_{i}")
            for t in range(2):
                hi = hbuf[i + t]
                for j in range(ND):
                    nc.tensor.transpose(
                        tp[:, t, j, :], hi[:, j * 128:(j + 1) * 128], ident[:]
                    )
            # dst [128, ND, 256] <- src reordered (j, t)
            src = tp[:].rearrange("p t j c -> p j t c")
            nc.vector.tensor_copy(dstT[:, :, i * 128:(i + 2) * 128].rearrange(
                "p j (t c) -> p j t c", c=128), src)

        hT_next = None
        wqv = None

        def v_gemm(tiles, v, wqv):
            for i in tiles:
                vv = v[:, i, :].rearrange("p (h e) -> p h e", e=65)
                nc.gpsimd.memset(vv[:, :, 64:65], AV_ / AO)
                pt = psB.tile([128, 2, 512], f32, tag="wk", name=f"vps_{i}")
                for j2 in range(ND2):
                    for c in range(2):
                        nc.tensor.matmul(
                            pt[:, c, :],
                            hT[:, 2 * j2:2 * j2 + 2, i * 128:(i + 1) * 128],
                            wqv[:, 2 * j2:2 * j2 + 2, c * 512:(c + 1) * 512],
                            start=(j2 == 0),
                            stop=(j2 == ND2 - 1),
                            perf_mode=DR,
                        )
                nc.vector.tensor_scalar_mul(
                    vv[:, :, 0:64].rearrange("p (c h) e -> p c h e", c=2),
                    pt[:].rearrange("p c (h e) -> p c h e", e=64),
                    AV_ * DQ_H,
                )

        for l in range(nl):
            # ================= attention =================
            if l == 0:
                hT = htpool.tile([128, ND, L], f8, tag="hT")
                hbuf = {}
                wqv = w16.tile([128, ND, 1024], f8, tag="w16", name="wqv0")
                nc.sync.dma_start(wqv[:], wqv_d[0])
                v = big.tile([128, NT, 16 * 65], f8, tag="big", name="v0")
                for pr in range(4):
                    i0 = 2 * pr
                    ln_group((i0, i0 + 1), hbuf, epsa, 1.0 / (AH * AH),
                             f"i{i0}")
                    lnT_pair(hbuf, i0, hT)
                    v_gemm((i0, i0 + 1), v, wqv)
            else:
                hT = hT_next
                v = v_pending

            if has_qb:
                bq_sb = bpool.tile([128, ND], f32, tag="bq")
                nc.sync.dma_start(bq_sb[:], bq_d[l])

            if l != 0:
                v_gemm(range(4, 8), v, wqv)

            # wo prefetch (w16 slot frees once wqv is released after V GEMM)
            wo_sb = w16.tile([128, ND2, 2, 1024], f8, tag="w16", name=f"wo{l}")
            nc.sync.dma_start(wo_sb[:], wo_d[l])
            if has_ob:
                bo_sb = bpool.tile([128, D], f16, tag="bo")
                nc.sync.dma_start(bo_sb[:], bo_d[l])

            qT = big.tile([128, ND, 1040], f16, tag="big", name=f"qT{l}")
            kT = big.tile([128, ND, 1040], f16, tag="big", name=f"kT{l}")
            oT = big.tile([128, ND, 1040], f8, tag="big", name=f"oT{l}")

            def qk_tile(m):
                wt = wsp.tile([128, ND2, 2, 128], f8, tag="ws", name=f"wt_{l}_{m}")
                nc.sync.dma_start(wt[:], wqk_d[l, m])
                dst = qT if m < ND else kT
                pt = psB.tile([128, 2, 512], f32, tag="wk", name=f"qkps_{l}_{m}")
                for j2 in range(ND2):
                    for c in range(2):
                        nc.tensor.matmul(
                            pt[:, c, :],
                            wt[:, j2, :, :],
                            hT[:, 2 * j2:2 * j2 + 2, c * 512:(c + 1) * 512],
                            start=(j2 == 0),
                            stop=(j2 == ND2 - 1),
                            perf_mode=DR,
                        )
                if m < ND and has_qb:  # Q bias (per-partition)
                    nc.vector.tensor_scalar(
                        dst[:, m, 0:1024], pt[:].rearrange("p c q -> p (c q)"),
                        DQ_H, bq_sb[:, m:m + 1], OP.mult, OP.add)
                else:
                    nc.vector.tensor_scalar_mul(
                        dst[:, m % ND, 0:1024],
                        pt[:].rearrange("p c q -> p (c q)"), DQ_H)

            def score_j(jo, c, j, aT):
                """Score MMs + merged exp for one key tile j (both heads)."""
                w0 = max(0, (j - 4) * 128) if c == 1 else 0
                diag = (c == 1 and j >= 4)
                sc = psC.tile([128, 2, 512], f32, tag="sc",
                              name=f"sc_{l}_{jo}_{c}_{j}")
                for pi, po in enumerate((0, 64)):
                    nc.tensor.matmul(
                        sc[:, pi, w0:512],
                        kT[po:po + 64, jo, j * 128:(j + 1) * 128],
                        qT[po:po + 64, jo, c * 512 + w0:(c + 1) * 512],
                        start=True, stop=not diag,
                    )
                if diag:
                    # additive -3e4 on the strict upper triangle of the
                    # diagonal block, via a const matmul into the same PSUM
                    for pi in (0, 1):
                        nc.tensor.matmul(
                            sc[:, pi, w0:w0 + 128], mbias[:], ident[:],
                            start=False, stop=True,
                        )
                nc.scalar.activation(aT[:, :, j, w0:512], sc[:, :, w0:512],
                                     AF.Exp, scale=0.125)
                # zero the strips the AV pair reads but exp never writes
                if c == 1 and j == 5:
                    nc.gpsimd.memset(aT[:, :, 5, 0:128], 0.0)
                if c == 1 and j == 7:
                    nc.gpsimd.memset(aT[:, :, 7, 256:384], 0.0)

            def av_pairs(jo, c, aT, us):
                """AV DoubleRow accumulation for pair indices us (list)."""
                nkt = 4 if c == 0 else 8
                np2 = nkt // 2
                for u in us:
                    j = 2 * u
                    w0 = max(0, (j - 4) * 128)
                    for pi in (0, 1):
                        hh = 2 * jo + pi
                        nc.tensor.matmul(
                            opsl[0:65, pi, w0:512],
                            v[:, j:j + 2, 65 * hh:65 * hh + 65],
                            aT[:, pi, j:j + 2, w0:512],
                            start=(u == 0),
                            stop=(u == np2 - 1),
                            perf_mode=DR,
                        )

            def z_chain(jo, c):
                for pi, po in enumerate((0, 64)):
                    rz = rzp.tile([1, 512], f32, tag="rz",
                                  name=f"rz_{l}_{jo}_{c}_{pi}")
                    bcs = bcp.tile([64, 512], f32, tag="bcs",
                                   name=f"bc_{l}_{jo}_{c}_{pi}")
                    if GPB:
                        nc.vector.tensor_copy(rz[:], opsl[64:65, pi, :])
                        nc.vector.reciprocal_approx_fast(rz[:], rz[:])
                        nc.gpsimd.partition_broadcast(bcs[:], rz[:], channels=64)
                    else:
                        # v2-style: bounce Z through DRAM to broadcast, then recip
                        nc.vector.tensor_copy(rz[:], opsl[64:65, pi, :])
                        rzd = dpool.tile([512], f32, tag="rzd",
                                         name=f"rzd_{l}_{jo}_{c}_{pi}")
                        nc.sync.dma_start(rzd[:].unsqueeze(0), rz[:])
                        nc.sync.dma_start(
                            bcs[:], rzd[:].unsqueeze(0).broadcast_to((64, 512)))
                        nc.vector.reciprocal_approx_fast(bcs[:], bcs[:])
                    nc.vector.tensor_mul(
                        oT[po:po + 64, jo, c * 512:(c + 1) * 512],
                        opsl[0:64, pi, :], bcs[:]
                    )

            def dbg_dump_t(src_ap, j, cols=1024):
                xo = f4k.tile([128, D], f32, tag="xo", name=f"dbg_{l}_{j}")
                nc.vector.tensor_copy(xo[:, 0:cols], src_ap)
                nc.sync.dma_start(out_d[j * 128:(j + 1) * 128, 0:cols], xo[:, 0:cols])

            if DBG == "hT" and l == 0:
                for j in range(ND):
                    dbg_dump_t(hT[:, j, 0:1024], j)
                break
            if DBG == "v" and l == 0:
                for i in range(NT):
                    dbg_dump_t(v[:, i, 0:1024], i)
                break

            qk_tile(0)
            qk_tile(ND)
            if DBG in ("qT", "kT") and l == 0:
                for m in range(1, ND):
                    qk_tile(m)
                    qk_tile(ND + m)
                src = qT if DBG == "qT" else kT
                for j in range(ND):
                    dbg_dump_t(src[:, j, 0:1024], j)
                break
            for jo in range(ND):
                aT0 = apool.tile([128, 2, 8, 512], f8, tag="aT",
                                 name=f"aT_{l}_{jo}_0")
                aT1 = apool.tile([128, 2, 8, 512], f8, tag="aT",
                                 name=f"aT_{l}_{jo}_1")
                score_j(jo, 0, 0, aT0)
                score_j(jo, 0, 1, aT0)
                if jo < ND - 1:
                    qk_tile(jo + 1)
                score_j(jo, 0, 2, aT0)
                score_j(jo, 0, 3, aT0)
                opsl = psB.tile([128, 2, 512], f32, tag="wk",
                                name=f"op_{l}_{jo}_0")
                av_pairs(jo, 0, aT0, (0,))
                score_j(jo, 1, 0, aT1)
                score_j(jo, 1, 1, aT1)
                av_pairs(jo, 0, aT0, (1,))
                z_chain(jo, 0)
                if jo < ND - 1:
                    qk_tile(ND + jo + 1)
                score_j(jo, 1, 2, aT1)
                score_j(jo, 1, 3, aT1)
                opsl = psB.tile([128, 2, 512], f32, tag="wk",
                                name=f"op_{l}_{jo}_1")
                av_pairs(jo, 1, aT1, (0,))
                score_j(jo, 1, 4, aT1)
                score_j(jo, 1, 5, aT1)
                av_pairs(jo, 1, aT1, (1,))
                score_j(jo, 1, 6, aT1)
                score_j(jo, 1, 7, aT1)
                av_pairs(jo, 1, aT1, (2,))
                av_pairs(jo, 1, aT1, (3,))
                z_chain(jo, 1)
                if DBG == "aT" and l == 0 and jo == 0:
                    for j in range(8):
                        dbg_dump_t(aT0[:, 0, j, :], j, cols=512)
                    for j in range(8):
                        xo = f4k.tile([128, D], f32, tag="xo", name=f"dbgb_{j}")
                        nc.vector.tensor_copy(xo[:, 0:512], aT1[:, 0, j, :])
                        nc.sync.dma_start(out_d[j * 128:(j + 1) * 128, 512:1024],
                                          xo[:, 0:512])
                    break

            if DBG == "aT" and l == 0:
                break
            if DBG == "oT" and l == 0:
                for j in range(ND):
                    dbg_dump_t(oT[:, j, 0:1024], j)
                break

            # out-proj + residual fused in PSUM-evac, then LN2 per 4-tile
            # group so the DVE work overlaps the PE
            h2T = htpool.tile([128, ND, L], f8 if FFN8 else f16, tag="hT",
                              name=f"h2T{l}")
            h2buf = {}

            def out_proj_mm(i):
                yp = psB.tile([128, 2, 512], f32, tag="wk", name=f"ops_{i}")
                for j2 in range(ND2):
                    for c in range(2):
                        nc.tensor.matmul(
                            yp[:, c, :],
                            oT[:, 2 * j2:2 * j2 + 2, i * 128:(i + 1) * 128],
                            wo_sb[:, j2, :, c * 512:(c + 1) * 512],
                            start=(j2 == 0),
                            stop=(j2 == ND2 - 1),
                            perf_mode=DR,
                        )
                return yp

            def out_proj_ev(i, yp):
                # x += psum/(AO*SW)  (one fused DVE op)
                nc.vector.scalar_tensor_tensor(
                    x[:, i, :], yp[:].rearrange("p c q -> p (c q)"),
                    1.0 / (AO * SW), x[:, i, :], OP.mult, OP.add)
                if has_ob:
                    nc.vector.tensor_add(x[:, i, :], x[:, i, :], bo_sb[:])

            def out_proj(tiles, ev=True):
                yps = []
                for i in tiles:
                    yp = out_proj_mm(i)
                    if ev:
                        out_proj_ev(i, yp)
                    else:
                        yps.append((i, yp))
                return yps

            ep2 = epsa if FFN8 else epst
            iv2 = 1.0 / (AH * AH) if FFN8 else 1.0
            out_proj((0, 1))
            ln_group((0, 1), h2buf, ep2, iv2, f"a{l}")
            out_proj((2, 3))
            ln_group((2, 3), h2buf, ep2, iv2, f"b{l}")
            out_proj((4, 5))
            lnT_pair(h2buf, 0, h2T)
            ln_group((4, 5), h2buf, ep2, iv2, f"c{l}")
            out_proj((6, 7))
            lnT_pair(h2buf, 2, h2T)
            ln_group((6, 7), h2buf, ep2, iv2, f"d{l}")
            lnT_pair(h2buf, 4, h2T)
            lnT_pair(h2buf, 6, h2T)

            if DBG == "xattn" and l == 0:
                lnT_pair(h2buf, 4, h2T)
                lnT_pair(h2buf, 6, h2T)
                for i in range(NT):
                    dbg_dump_t(x[:, i, :], i)
                break

            # ================= FFN =================
            if has_b1:
                b1_sb = bpool.tile([128, NF], f32, tag="b1")
                nc.sync.dma_start(b1_sb[:], b1_d[l])
            if has_b2:
                b2_sb = bpool.tile([128, D], f16, tag="b2")
                nc.sync.dma_start(b2_sb[:], b2_d[l])

            # wqv for next layer (slot frees after this layer's out_proj)
            if l != nl - 1:
                wqv = w16.tile([128, ND, 1024], f8, tag="w16", name=f"wqv{l+1}")
                nc.sync.dma_start(wqv[:], wqv_d[l + 1])

            # GEMM1 (+ gelu) -> g^T [ff(P), t] quarters; c-halves split so
            # the c=0 GEMMs start as soon as token tiles 0-3 are transposed
            gq = []
            for q in range(4):
                g = big.tile([128, 8, 1040], f8 if FFN8 else f16, tag="big",
                             name=f"gT_{l}_{q}")
                gq.append(g)

            def ffn1_all():
                for f in range(NF):
                    gp = psB.tile([128, 2, 512], f32, tag="wk", name=f"g1_{f}")
                    if FFN8:
                        w1t = wsp.tile([128, ND2, 2, 128], f8, tag="ws",
                                       name=f"w1_{l}_{f}")
                        nc.sync.dma_start(w1t[:], w1_d[l, f])
                        for j2 in range(ND2):
                            for c in range(2):
                                nc.tensor.matmul(
                                    gp[:, c, :],
                                    w1t[:, j2, :, :],
                                    h2T[:, 2 * j2:2 * j2 + 2,
                                        c * 512:(c + 1) * 512],
                                    start=(j2 == 0),
                                    stop=(j2 == ND2 - 1),
                                    perf_mode=DR,
                                )
                    else:
                        w1t = wsp.tile([128, ND, 128], f16, tag="ws",
                                       name=f"w1_{l}_{f}")
                        nc.sync.dma_start(w1t[:], w1_d[l, f])
                        for j in range(ND):
                            for c in range(2):
                                nc.tensor.matmul(
                                    gp[:, c, :],
                                    w1t[:, j, :],
                                    h2T[:, j, c * 512:(c + 1) * 512],
                                    start=(j == 0),
                                    stop=(j == ND - 1),
                                )
                    gsc = GDQ if FFN8 else 1.0
                    if has_b1:
                        nc.scalar.activation(
                            gq[f // 8][:, f % 8, 0:1024],
                            gp[:].rearrange("p c q -> p (c q)"),
                            AF.Gelu, bias=b1_sb[:, f:f + 1], scale=gsc)
                    else:
                        nc.scalar.activation(
                            gq[f // 8][:, f % 8, 0:1024],
                            gp[:].rearrange("p c q -> p (c q)"),
                            AF.Gelu, scale=gsc)

            ffn1_all()

            # GEMM2: acc pairs (2 t-tiles per 2-bank slot), w2 streamed
            last = nl - 1
            hT_next = None if l == last else htpool.tile(
                [128, ND, L], f8, tag="hT", name=f"hTn_{l}")
            hnbuf = {}

            def ffn2(tg):
                for c in range(2):
                    cs = slice(c * 512, (c + 1) * 512)
                    ys = [psB.tile([128, 2, 512], f32, tag="wk",
                                   name=f"psy_{l}_{c}_{tg[0]}_{k}")
                          for k in range((len(tg) + 1) // 2)]
                    if FFN8:
                        for f2 in range(NF2):
                            w2t = w2p.tile([128, 2, 512], f8, tag="w2s")
                            nc.sync.dma_start(w2t[:], w2_d[l, c, f2])
                            q, fo = f2 // 4, 2 * (f2 % 4)
                            for k, i in enumerate(tg):
                                nc.tensor.matmul(
                                    ys[k // 2][:, k % 2, :],
                                    gq[q][:, fo:fo + 2, i * 128:(i + 1) * 128],
                                    w2t[:],
                                    start=(f2 == 0),
                                    stop=(f2 == NF2 - 1),
                                    perf_mode=DR,
                                )
                        dq = 1.0 / SW2
                    else:
                        for f2 in range(NF2):
                            w2t = w2p.tile([128, 2, 512], f16, tag="w2s")
                            nc.sync.dma_start(w2t[:], w2_d[l, c, f2])
                            for ff in range(2):
                                f = 2 * f2 + ff
                                for k, i in enumerate(tg):
                                    nc.tensor.matmul(
                                        ys[k // 2][:, k % 2, :],
                                        gq[f // 8][:, f % 8,
                                           i * 128:(i + 1) * 128],
                                        w2t[:, ff, :],
                                        start=(f == 0),
                                        stop=(f == NF - 1),
                                    )
                        dq = 1.0
                    for k, i in enumerate(tg):
                        nc.vector.scalar_tensor_tensor(
                            x[:, i, cs], ys[k // 2][:, k % 2, :], dq,
                            x[:, i, cs], OP.mult, OP.add)
                        if has_b2:
                            nc.vector.tensor_add(x[:, i, cs], x[:, i, cs],
                                                 b2_sb[:, cs])

            epn = epsa
            ivn = 1.0 / (AH * AH)
            if l != last:
                ffn2((0, 1, 2, 3))
                ln_group(range(0, 4), hnbuf, epn, ivn, f"n0{l}")
                ffn2((4, 5))
                ln_group((4, 5), hnbuf, epn, ivn, f"n4{l}")
                lnT_pair(hnbuf, 0, hT_next)
                lnT_pair(hnbuf, 2, hT_next)
                ffn2((6, 7))
                ln_group((6, 7), hnbuf, epn, ivn, f"n6{l}")
                v_pending = big.tile([128, NT, 16 * 65], f8, tag="big",
                                     name=f"v{l+1}")
                # early V GEMM for next layer (tiles 0-3) while DVE does LN
                hT = hT_next
                v_gemm(range(0, 4), v_pending, wqv)
                lnT_pair(hnbuf, 4, hT_next)
                lnT_pair(hnbuf, 6, hT_next)
            else:
                # final layer: fuse the final LN + store into the FFN2 tail
                flns = f4k.tile([128, D], f32, tag="f4k")
                nc.sync.dma_start(flns[:], fs_d[:])
                flnb = f4k.tile([128, D], f32, tag="f4k")
                nc.sync.dma_start(flnb[:], fb_d[:])

                def fln_emit(tiles, suf):
                    fbuf = {}
                    ln_group(tiles, fbuf, epst, 1.0, suf)
                    for i in tiles:
                        xo = f4k.tile([128, D], f32, tag="xo", name=f"xo_{i}")
                        nc.vector.tensor_mul(xo[:], fbuf[i][:], flns[:])
                        nc.vector.tensor_add(xo[:], xo[:], flnb[:])
                        nc.sync.dma_start(out_d[i * 128:(i + 1) * 128, :], xo[:])

                ffn2((0, 1, 2, 3))
                fln_emit(range(0, 4), "f0")
                ffn2((4, 5))
                fln_emit((4, 5), "f4")
                ffn2((6, 7))
                fln_emit((6, 7), "f6")

    nc.compile()
    return nc


def _host_prep(sensor_tokens, traj_tokens, pos_embed, ln1_s, ln1_b,
               qkv_w, qkv_b, out_w, out_b, ln2_s, ln2_b,
               w1, b1, w2, b2, fln_s, fln_b, nl=NL):
    """Fold LN affine params into weights; retile + fp8-cast with scaling."""
    fp = np.float32
    x_all = np.concatenate([sensor_tokens, traj_tokens], axis=1).astype(fp)
    x_all = x_all + pos_embed[:L][None].astype(fp)

    wqk8 = np.empty((nl, 16, 128, ND2, 2, 128), F8NP)
    wqv8 = np.empty((nl, 128, ND, 1024), F8NP)
    bqh = np.empty((nl, 128, ND), fp)
    wo8 = np.empty((nl, 128, ND2, 2, 1024), F8NP)
    boh = np.empty((nl, 128, D), np.float16)
    if FFN8:
        w1T = np.empty((nl, NF, 128, ND2, 2, 128), F8NP)
        w2T = np.empty((nl, 2, NF2, 128, 2, 512), F8NP)
    else:
        w1T = np.empty((nl, NF, 128, ND, 128), np.float16)
        w2T = np.empty((nl, 2, NF2, 128, 2, 512), np.float16)
    b1h = np.empty((nl, 128, NF), fp)
    b2h = np.empty((nl, 128, D), np.float16)

    for i in range(nl):
        Wq = qkv_w[i].astype(fp)                                  # [3D, D]
        bfull = qkv_b[i].astype(fp) + Wq @ ln1_b[i].astype(fp)    # [3D]
        Wq = Wq * ln1_s[i].astype(fp)[None, :]
        WqT = (Wq.T * SW).astype(F8NP)                            # [D, 3D]
        qk = WqT[:, :2 * D].reshape(ND2, 2, 128, 16, 128)
        wqk8[i] = qk.transpose(3, 2, 0, 1, 4)
        wqv8[i] = WqT[:, 2 * D:].reshape(ND, 128, 1024).transpose(1, 0, 2)
        bqh[i] = bfull[:D].reshape(ND, 128).T
        bv = bfull[2 * D:]
        Wo = out_w[i].astype(fp)                                  # [D, D]
        bo = out_b[i].astype(fp) + Wo @ bv
        wo8[i] = (Wo.T * SW).astype(F8NP).reshape(ND2, 2, 128, 1024).transpose(2, 0, 1, 3)
        boh[i] = np.broadcast_to(bo.astype(np.float16), (128, D))
        W1 = w1[i].astype(fp)                                     # [FF, D]
        b1f = b1[i].astype(fp) + W1 @ ln2_b[i].astype(fp)
        W1 = W1 * ln2_s[i].astype(fp)[None, :]
        if FFN8:
            W1t = (W1.T * SW1).astype(F8NP)                       # [D, FF]
            # [d, ff] -> (f, p(ffchunk? no: p=d%128), j2, jj, col)
            w1T[i] = W1t.reshape(ND2, 2, 128, NF, 128).transpose(3, 2, 0, 1, 4)
            W2t = (w2[i].astype(fp).T * SW2).astype(F8NP)         # [FF, D]
            w2T[i] = W2t.reshape(NF2, 2, 128, 2, 512).transpose(3, 0, 2, 1, 4)
        else:
            W1t = W1.T.astype(np.float16)                         # [D, FF]
            w1T[i] = W1t.reshape(ND, 128, NF, 128).transpose(2, 1, 0, 3)
            W2t = w2[i].astype(fp).T.astype(np.float16)           # [FF, D]
            w2T[i] = W2t.reshape(NF2, 2, 128, 2, 512).transpose(3, 0, 2, 1, 4)
        b1h[i] = b1f.reshape(NF, 128).T
        b2h[i] = np.broadcast_to(b2[i].astype(np.float16), (128, D))

    common = dict(
        wqk=wqk8, wqv=wqv8, bq=bqh, wo=wo8, bo=boh,
        w1=w1T, b1=b1h, w2=w2T, b2=b2h,
        flns=np.broadcast_to(fln_s.astype(fp), (128, D)).copy(),
        flnb=np.broadcast_to(fln_b.astype(fp), (128, D)).copy(),
        ident=np.eye(128, dtype=np.float16),
        mbias=np.triu(np.full((128, 128), -30000.0, np.float16), 1),
    )
    in_maps = [dict(common, x0=np.ascontiguousarray(x_all[c])) for c in range(B)]
    return in_maps


_NC = {}
LAST_RESULT = None


def kernel(**inputs):
    global LAST_RESULT
    in_maps = _host_prep(**inputs)
    m0 = in_maps[0]
    flags = (bool(np.any(m0["bq"])), bool(np.any(m0["bo"])),
             bool(np.any(m0["b1"])), bool(np.any(m0["b2"])))
    if flags not in _NC:
        _NC[flags] = build_nc(NL, *flags)
    res = run_bass_kernel_spmd(_NC[flags], in_maps, core_ids=list(range(B)))
    LAST_RESULT = res
    return np.stack([res.results[c]["out"] for c in range(B)]).astype(np.float32)


# revision 22
# speedup vs baseline: 1.0779x; 1.0042x over previous
"""Trainium2 Bass kernel for a 12-layer prefix-causal transformer.

Sharding: data-parallel over batch B=8 across 8 NeuronCores (1 sequence per
core, weights replicated, no collectives).

v3 (from v2's fp8 DoubleRow attention baseline), trace-driven restructure:
  - scores exp merged into multi-bank PSUM groups: 6 big ACTIVATEs per
    head-pair instead of 24 small ones (scalar engine was the attention
    bottleneck at ~130us/layer; the 352-cycle per-instr overhead dominated)
  - QK/V evacuations moved from scalar (ACTIVATE Identity) to DVE
    tensor_scalar, so the scalar engine runs exp exclusively in attention
  - software pipelining: qk_tile(m+1) GEMMs are emitted between scores(m)
    and AV(m) so the PE has work while exp runs
  - softmax 1/Z row broadcast via gpsimd partition_broadcast instead of a
    DRAM round-trip
  - residual adds fused into PSUM-evacuation scalar_tensor_tensor on DVE
    (the identity-matmul trick is gone; frees ~7us/layer of PE)
  - cross-layer pipeline: next layer's V GEMM is emitted during the tail
    LN/transpose so the PE never drains at layer boundaries (HAM stays warm)
  - optional fp8 DoubleRow FFN (BASS_FFN8=1): both FFN GEMMs at 2x PE rate;
    h2/g activations quantized e4m3 (validated against a numpy quant-sim of
    the full 12-layer error accumulation)
"""

import os
import sys
from contextlib import ExitStack

for _p in ("/opt/trn_rl_repo", "/root/.axon_site/_ro/trn_rl_repo"):
    if os.path.isdir(_p) and _p not in sys.path:
        sys.path.insert(0, _p)

import numpy as np
import ml_dtypes

import concourse.bass as bass
import concourse.tile as tile
from concourse import bacc, mybir
from concourse.bass_utils import run_bass_kernel_spmd

dt = mybir.dt
AF = mybir.ActivationFunctionType
OP = mybir.AluOpType
DR = mybir.MatmulPerfMode.DoubleRow

B, LS = 8, 512
L, D, H, HD, FF = 1024, 1024, 16, 64, 4096
NL = int(os.environ.get("BASS_NL", "12"))
FFN8 = os.environ.get("BASS_FFN8", "0") == "1"
DBG = os.environ.get("BASS_DBG", "")
GPB = os.environ.get("BASS_GPB", "1") == "1"
NT = L // 128    # 8 token tiles
ND = D // 128    # 8 d_model tiles
ND2 = ND // 2    # 4 DoubleRow k-pairs
NF = FF // 128   # 32 ffn tiles
NF2 = NF // 2    # 16 ffn DoubleRow pairs
EPS = 1e-5

SW = 64.0        # attention weight scale
AH = 4.0         # LN-output activation scale (hT, h2T)
AV_ = 4.0        # v scale
AO = 4.0         # o scale (must equal AV_ for the free ones-trick requant)
DQ_H = 1.0 / (SW * AH)          # psum dequant for h-GEMMs: 1/256
SW1 = 64.0       # w1 fp8 scale
SW2 = 64.0       # w2 fp8 scale

f32, f16, f8 = dt.float32, dt.float16, dt.float8e4
F8NP = ml_dtypes.float8_e4m3


def build_nc(nl=NL, has_qb=True, has_ob=True, has_b1=True, has_b2=True):
    nc = bacc.Bacc("TRN2", target_bir_lowering=False, debug=False, num_devices=8)

    x_d = nc.dram_tensor("x0", [L, D], f32, kind="ExternalInput")
    wqk_d = nc.dram_tensor("wqk", [nl, 16, 128, ND2, 2, 128], f8, kind="ExternalInput")
    wqv_d = nc.dram_tensor("wqv", [nl, 128, ND, 1024], f8, kind="ExternalInput")
    bq_d = nc.dram_tensor("bq", [nl, 128, ND], f32, kind="ExternalInput")
    wo_d = nc.dram_tensor("wo", [nl, 128, ND2, 2, 1024], f8, kind="ExternalInput")
    bo_d = nc.dram_tensor("bo", [nl, 128, D], f16, kind="ExternalInput")
    if FFN8:
        w1_d = nc.dram_tensor("w1", [nl, NF, 128, ND2, 2, 128], f8, kind="ExternalInput")
    else:
        w1_d = nc.dram_tensor("w1", [nl, NF, 128, ND, 128], f16, kind="ExternalInput")
    b1_d = nc.dram_tensor("b1", [nl, 128, NF], f32, kind="ExternalInput")
    if FFN8:
        w2_d = nc.dram_tensor("w2", [nl, 2, NF2, 128, 2, 512], f8, kind="ExternalInput")
    else:
        w2_d = nc.dram_tensor("w2", [nl, 2, NF2, 128, 2, 512], f16, kind="ExternalInput")
    b2_d = nc.dram_tensor("b2", [nl, 128, D], f16, kind="ExternalInput")
    fs_d = nc.dram_tensor("flns", [128, D], f32, kind="ExternalInput")
    fb_d = nc.dram_tensor("flnb", [128, D], f32, kind="ExternalInput")
    id_d = nc.dram_tensor("ident", [128, 128], f16, kind="ExternalInput")
    mk_d = nc.dram_tensor("mbias", [128, 128], f16, kind="ExternalInput")
    out_d = nc.dram_tensor("out", [L, D], f32, kind="ExternalOutput")

    GDQ = 1.0 / (SW1 * AH)   # FFN1 psum dequant (h2T carries AH)

    with tile.TileContext(nc) as tc, ExitStack() as ctx:
        def pool(name, bufs, space="SBUF"):
            return ctx.enter_context(tc.tile_pool(name=name, bufs=bufs, space=space))

        cpool = pool("consts", 1)       # ident, mask2, eps tiles
        small = pool("small", 8)        # LN stats
        rzp = pool("rz", 3)             # [1,512] f32 recip of Z
        bcp = pool("bcs", 3)            # [64,512] f32 broadcast recip
        xpool = pool("x", 1)            # residual fp16, 16KB
        f4k = pool("f4k", 2)            # x-load + final-LN consts, f32 4KB
        hpool = pool("h", 6)            # LN output per t-tile, f16 2KB
        htpool = pool("hT", 2)          # transposed LN output fp8, 8KB
        big = pool("big", 4)            # qT,kT (f16) / v,oT + 4 gT
        apool = pool("aT", 3)           # exp(scores) fp8 [128,2,8,512], 8KB
        wsp = pool("wstream", 3)        # streamed QK/W1 weight tiles
        w16 = pool("w16", 1)            # V-part / out-proj weights fp8 8KB
        w2p = pool("w2s", 6)            # streamed GEMM2 weight tiles
        bpool = pool("bias", 1)         # per-layer bias tiles
        dpool = pool("dscr", 4, space="DRAM")  # 1/Z DRAM bounce (GPB=0 path)
        psC = pool("psC", 2, space="PSUM")   # scores: 2x [128,2,512]
        psB = pool("psB", 2, space="PSUM")   # everything else: 2x 2 banks

        ident = cpool.tile([128, 128], f16)
        nc.sync.dma_start(ident[:], id_d[:])
        mbias = cpool.tile([128, 128], f16)
        nc.sync.dma_start(mbias[:], mk_d[:])
        epsa = cpool.tile([128, 1], f32)
        nc.gpsimd.memset(epsa[:], EPS / (AH * AH))
        epst = cpool.tile([128, 1], f32)
        nc.gpsimd.memset(epst[:], EPS)

        # ---- initial x = (sensor|traj + pos), cast to fp16 ----
        x = xpool.tile([128, NT, D], f16)
        for i in range(NT):
            xt = f4k.tile([128, D], f32, tag="f4k")
            nc.sync.dma_start(xt[:], x_d[i * 128:(i + 1) * 128, :])
            nc.vector.tensor_copy(x[:, i, :], xt[:])

        def ln_group(tiles, hbuf, epsv, inva, tagsuf=""):
            """LN stats+normalize for a group of t-tiles; one batched sqrt."""
            ntl = len(tiles)
            mvb = small.tile([128, ntl, 2], f32, tag="mvb", name=f"mvb{tagsuf}")
            for t, i in enumerate(tiles):
                st = small.tile([128, 12], f32, tag="stats")
                nc.vector.bn_stats(st[:, 0:6], x[:, i, 0:512])
                nc.vector.bn_stats(st[:, 6:12], x[:, i, 512:1024])
                nc.vector.bn_aggr(mvb[:, t, :], st[:])
            stdb = small.tile([128, ntl], f32, tag="stdb", name=f"stdb{tagsuf}")
            nc.scalar.activation(stdb[:], mvb[:, :, 1], AF.Sqrt,
                                 bias=epsv[:], scale=inva)
            rstdb = small.tile([128, ntl], f32, tag="rstdb", name=f"rstdb{tagsuf}")
            nc.vector.reciprocal_approx_fast(rstdb[:], stdb[:])
            nmrb = small.tile([128, ntl], f32, tag="nmrb", name=f"nmrb{tagsuf}")
            nc.vector.tensor_mul(nmrb[:], mvb[:, :, 0], rstdb[:])
            for t, i in enumerate(tiles):
                hi = hpool.tile([128, D], f16, tag="h", name=f"h{tagsuf}_{i}")
                # h = x*rstd - mean*rstd
                nc.vector.tensor_scalar(hi[:], x[:, i, :], rstdb[:, t:t + 1],
                                        nmrb[:, t:t + 1], OP.mult, OP.subtract)
                hbuf[i] = hi

        def lnT_pair(hbuf, i, dstT):
            """PE-transpose tiles i, i+1 into dstT columns; one DVE evac."""
            tp = psB.tile([128, 2, ND, 128], f16, tag="wk", name=f"tp_{i}")
            for t in range(2):
                hi = hbuf[i + t]
                for j in range(ND):
                    nc.tensor.transpose(
                        tp[:, t, j, :], hi[:, j * 128:(j + 1) * 128], ident[:]
                    )
            # dst [128, ND, 256] <- src reordered (j, t)
            src = tp[:].rearrange("p t j c -> p j t c")
            nc.vector.tensor_copy(dstT[:, :, i * 128:(i + 2) * 128].rearrange(
                "p j (t c) -> p j t c", c=128), src)

        hT_next = None
        wqv = None

        def v_gemm(tiles, v, wqv):
            for i in tiles:
                vv = v[:, i, :].rearrange("p (h e) -> p h e", e=65)
                nc.gpsimd.memset(vv[:, :, 64:65], AV_ / AO)
                pt = psB.tile([128, 2, 512], f32, tag="wk", name=f"vps_{i}")
                for j2 in range(ND2):
                    for c in range(2):
                        nc.tensor.matmul(
                            pt[:, c, :],
                            hT[:, 2 * j2:2 * j2 + 2, i * 128:(i + 1) * 128],
                            wqv[:, 2 * j2:2 * j2 + 2, c * 512:(c + 1) * 512],
                            start=(j2 == 0),
                            stop=(j2 == ND2 - 1),
                            perf_mode=DR,
                        )
                nc.vector.tensor_scalar_mul(
                    vv[:, :, 0:64].rearrange("p (c h) e -> p c h e", c=2),
                    pt[:].rearrange("p c (h e) -> p c h e", e=64),
                    AV_ * DQ_H,
                )

        for l in range(nl):
            # ================= attention =================
            if l == 0:
                hT = htpool.tile([128, ND, L], f8, tag="hT")
                hbuf = {}
                wqv = w16.tile([128, ND, 1024], f8, tag="w16", name="wqv0")
                nc.sync.dma_start(wqv[:], wqv_d[0])
                v = big.tile([128, NT, 16 * 65], f8, tag="big", name="v0")
                for pr in range(4):
                    i0 = 2 * pr
                    ln_group((i0, i0 + 1), hbuf, epsa, 1.0 / (AH * AH),
                             f"i{i0}")
                    lnT_pair(hbuf, i0, hT)
                    v_gemm((i0, i0 + 1), v, wqv)
            else:
                hT = hT_next
                v = v_pending

            if has_qb:
                bq_sb = bpool.tile([128, ND], f32, tag="bq")
                nc.sync.dma_start(bq_sb[:], bq_d[l])

            if l != 0:
                v_gemm(range(4, 8), v, wqv)

            # wo prefetch (w16 slot frees once wqv is released after V GEMM)
            wo_sb = w16.tile([128, ND2, 2, 1024], f8, tag="w16", name=f"wo{l}")
            nc.sync.dma_start(wo_sb[:], wo_d[l])
            if has_ob:
                bo_sb = bpool.tile([128, D], f16, tag="bo")
                nc.sync.dma_start(bo_sb[:], bo_d[l])

            qT = big.tile([128, ND, 1040], f16, tag="big", name=f"qT{l}")
            kT = big.tile([128, ND, 1040], f16, tag="big", name=f"kT{l}")
            oT = big.tile([128, ND, 1040], f8, tag="big", name=f"oT{l}")

            def qk_tile(m):
                wt = wsp.tile([128, ND2, 2, 128], f8, tag="ws", name=f"wt_{l}_{m}")
                nc.sync.dma_start(wt[:], wqk_d[l, m])
                dst = qT if m < ND else kT
                pt = psB.tile([128, 2, 512], f32, tag="wk", name=f"qkps_{l}_{m}")
                for j2 in range(ND2):
                    for c in range(2):
                        nc.tensor.matmul(
                            pt[:, c, :],
                            wt[:, j2, :, :],
                            hT[:, 2 * j2:2 * j2 + 2, c * 512:(c + 1) * 512],
                            start=(j2 == 0),
                            stop=(j2 == ND2 - 1),
                            perf_mode=DR,
                        )
                if m < ND and has_qb:  # Q bias (per-partition)
                    nc.vector.tensor_scalar(
                        dst[:, m, 0:1024], pt[:].rearrange("p c q -> p (c q)"),
                        DQ_H, bq_sb[:, m:m + 1], OP.mult, OP.add)
                else:
                    nc.vector.tensor_scalar_mul(
                        dst[:, m % ND, 0:1024],
                        pt[:].rearrange("p c q -> p (c q)"), DQ_H)

            def score_j(jo, c, j, aT):
                """Score MMs + merged exp for one key tile j (both heads)."""
                w0 = max(0, (j - 4) * 128) if c == 1 else 0
                diag = (c == 1 and j >= 4)
                sc = psC.tile([128, 2, 512], f32, tag="sc",
                              name=f"sc_{l}_{jo}_{c}_{j}")
                for pi, po in enumerate((0, 64)):
                    nc.tensor.matmul(
                        sc[:, pi, w0:512],
                        kT[po:po + 64, jo, j * 128:(j + 1) * 128],
                        qT[po:po + 64, jo, c * 512 + w0:(c + 1) * 512],
                        start=True, stop=not diag,
                    )
                if diag:
                    # additive -3e4 on the strict upper triangle of the
                    # diagonal block, via a const matmul into the same PSUM
                    for pi in (0, 1):
                        nc.tensor.matmul(
                            sc[:, pi, w0:w0 + 128], mbias[:], ident[:],
                            start=False, stop=True,
                        )
                nc.scalar.activation(aT[:, :, j, w0:512], sc[:, :, w0:512],
                                     AF.Exp, scale=0.125)
                # zero the strips the AV pair reads but exp never writes
                if c == 1 and j == 5:
                    nc.gpsimd.memset(aT[:, :, 5, 0:128], 0.0)
                if c == 1 and j == 7:
                    nc.gpsimd.memset(aT[:, :, 7, 256:384], 0.0)

            def av_pairs(jo, c, aT, us):
                """AV DoubleRow accumulation for pair indices us (list)."""
                nkt = 4 if c == 0 else 8
                np2 = nkt // 2
                for u in us:
                    j = 2 * u
                    w0 = max(0, (j - 4) * 128)
                    for pi in (0, 1):
                        hh = 2 * jo + pi
                        nc.tensor.matmul(
                            opsl[0:65, pi, w0:512],
                            v[:, j:j + 2, 65 * hh:65 * hh + 65],
                            aT[:, pi, j:j + 2, w0:512],
                            start=(u == 0),
                            stop=(u == np2 - 1),
                            perf_mode=DR,
                        )

            def z_chain(jo, c):
                for pi, po in enumerate((0, 64)):
                    rz = rzp.tile([1, 512], f32, tag="rz",
                                  name=f"rz_{l}_{jo}_{c}_{pi}")
                    bcs = bcp.tile([64, 512], f32, tag="bcs",
                                   name=f"bc_{l}_{jo}_{c}_{pi}")
                    if GPB:
                        nc.vector.tensor_copy(rz[:], opsl[64:65, pi, :])
                        nc.vector.reciprocal_approx_fast(rz[:], rz[:])
                        nc.gpsimd.partition_broadcast(bcs[:], rz[:], channels=64)
                    else:
                        # v2-style: bounce Z through DRAM to broadcast, then recip
                        nc.vector.tensor_copy(rz[:], opsl[64:65, pi, :])
                        rzd = dpool.tile([512], f32, tag="rzd",
                                         name=f"rzd_{l}_{jo}_{c}_{pi}")
                        nc.sync.dma_start(rzd[:].unsqueeze(0), rz[:])
                        nc.sync.dma_start(
                            bcs[:], rzd[:].unsqueeze(0).broadcast_to((64, 512)))
                        nc.vector.reciprocal_approx_fast(bcs[:], bcs[:])
                    nc.vector.tensor_mul(
                        oT[po:po + 64, jo, c * 512:(c + 1) * 512],
                        opsl[0:64, pi, :], bcs[:]
                    )

            def dbg_dump_t(src_ap, j, cols=1024):
                xo = f4k.tile([128, D], f32, tag="xo", name=f"dbg_{l}_{j}")
                nc.vector.tensor_copy(xo[:, 0:cols], src_ap)
                nc.sync.dma_start(out_d[j * 128:(j + 1) * 128, 0:cols], xo[:, 0:cols])

            if DBG == "hT" and l == 0:
                for j in range(ND):
                    dbg_dump_t(hT[:, j, 0:1024], j)
                break
            if DBG == "v" and l == 0:
                for i in range(NT):
                    dbg_dump_t(v[:, i, 0:1024], i)
                break

            qk_tile(0)
            qk_tile(ND)
            if DBG in ("qT", "kT") and l == 0:
                for m in range(1, ND):
                    qk_tile(m)
                    qk_tile(ND + m)
                src = qT if DBG == "qT" else kT
                for j in range(ND):
                    dbg_dump_t(src[:, j, 0:1024], j)
                break
            for jo in range(ND):
                aT0 = apool.tile([128, 2, 8, 512], f8, tag="aT",
                                 name=f"aT_{l}_{jo}_0")
                aT1 = apool.tile([128, 2, 8, 512], f8, tag="aT",
                                 name=f"aT_{l}_{jo}_1")
                score_j(jo, 0, 0, aT0)
                score_j(jo, 0, 1, aT0)
                if jo < ND - 1:
                    qk_tile(jo + 1)
                score_j(jo, 0, 2, aT0)
                score_j(jo, 0, 3, aT0)
                opsl = psB.tile([128, 2, 512], f32, tag="wk",
                                name=f"op_{l}_{jo}_0")
                av_pairs(jo, 0, aT0, (0,))
                score_j(jo, 1, 0, aT1)
                score_j(jo, 1, 1, aT1)
                av_pairs(jo, 0, aT0, (1,))
                z_chain(jo, 0)
                if jo < ND - 1:
                    qk_tile(ND + jo + 1)
                score_j(jo, 1, 2, aT1)
                score_j(jo, 1, 3, aT1)
                opsl = psB.tile([128, 2, 512], f32, tag="wk",
                                name=f"op_{l}_{jo}_1")
                av_pairs(jo, 1, aT1, (0,))
                score_j(jo, 1, 4, aT1)
                score_j(jo, 1, 5, aT1)
                av_pairs(jo, 1, aT1, (1,))
                score_j(jo, 1, 6, aT1)
                score_j(jo, 1, 7, aT1)
                av_pairs(jo, 1, aT1, (2,))
                av_pairs(jo, 1, aT1, (3,))
                z_chain(jo, 1)
                if DBG == "aT" and l == 0 and jo == 0:
                    for j in range(8):
                        dbg_dump_t(aT0[:, 0, j, :], j, cols=512)
                    for j in range(8):
                        xo = f4k.tile([128, D], f32, tag="xo", name=f"dbgb_{j}")
                        nc.vector.tensor_copy(xo[:, 0:512], aT1[:, 0, j, :])
                        nc.sync.dma_start(out_d[j * 128:(j + 1) * 128, 512:1024],
                                          xo[:, 0:512])
                    break

            if DBG == "aT" and l == 0:
                break
            if DBG == "oT" and l == 0:
                for j in range(ND):
                    dbg_dump_t(oT[:, j, 0:1024], j)
                break

            # out-proj + residual fused in PSUM-evac, then LN2 per 4-tile
            # group so the DVE work overlaps the PE
            h2T = htpool.tile([128, ND, L], f8 if FFN8 else f16, tag="hT",
                              name=f"h2T{l}")
            h2buf = {}

            def out_proj_mm(i):
                yp = psB.tile([128, 2, 512], f32, tag="wk", name=f"ops_{i}")
                for j2 in range(ND2):
                    for c in range(2):
                        nc.tensor.matmul(
                            yp[:, c, :],
                            oT[:, 2 * j2:2 * j2 + 2, i * 128:(i + 1) * 128],
                            wo_sb[:, j2, :, c * 512:(c + 1) * 512],
                            start=(j2 == 0),
                            stop=(j2 == ND2 - 1),
                            perf_mode=DR,
                        )
                return yp

            def out_proj_ev(i, yp):
                # x += psum/(AO*SW)  (one fused DVE op)
                nc.vector.scalar_tensor_tensor(
                    x[:, i, :], yp[:].rearrange("p c q -> p (c q)"),
                    1.0 / (AO * SW), x[:, i, :], OP.mult, OP.add)
                if has_ob:
                    nc.vector.tensor_add(x[:, i, :], x[:, i, :], bo_sb[:])

            def out_proj(tiles, ev=True):
                yps = []
                for i in tiles:
                    yp = out_proj_mm(i)
                    if ev:
                        out_proj_ev(i, yp)
                    else:
                        yps.append((i, yp))
                return yps

            ep2 = epsa if FFN8 else epst
            iv2 = 1.0 / (AH * AH) if FFN8 else 1.0
            out_proj((0, 1))
            ln_group((0, 1), h2buf, ep2, iv2, f"a{l}")
            out_proj((2, 3))
            ln_group((2, 3), h2buf, ep2, iv2, f"b{l}")
            out_proj((4, 5))
            lnT_pair(h2buf, 0, h2T)
            ln_group((4, 5), h2buf, ep2, iv2, f"c{l}")
            out_proj((6, 7))
            lnT_pair(h2buf, 2, h2T)
            ln_group((6, 7), h2buf, ep2, iv2, f"d{l}")
            lnT_pair(h2buf, 4, h2T)
            lnT_pair(h2buf, 6, h2T)

            if DBG == "xattn" and l == 0:
                lnT_pair(h2buf, 4, h2T)
                lnT_pair(h2buf, 6, h2T)
                for i in range(NT):
                    dbg_dump_t(x[:, i, :], i)
                break

            # ================= FFN =================
            if has_b1:
                b1_sb = bpool.tile([128, NF], f32, tag="b1")
                nc.sync.dma_start(b1_sb[:], b1_d[l])
            if has_b2:
                b2_sb = bpool.tile([128, D], f16, tag="b2")
                nc.sync.dma_start(b2_sb[:], b2_d[l])

            # wqv for next layer (slot frees after this layer's out_proj)
            if l != nl - 1:
                wqv = w16.tile([128, ND, 1024], f8, tag="w16", name=f"wqv{l+1}")
                nc.sync.dma_start(wqv[:], wqv_d[l + 1])

            # GEMM1 (+ gelu) -> g^T [ff(P), t] quarters; c-halves split so
            # the c=0 GEMMs start as soon as token tiles 0-3 are transposed
            gq = []
            for q in range(4):
                g = big.tile([128, 8, 1040], f8 if FFN8 else f16, tag="big",
                             name=f"gT_{l}_{q}")
                gq.append(g)

            def ffn1_all():
                for f in range(NF):
                    gp = psB.tile([128, 2, 512], f32, tag="wk", name=f"g1_{f}")
                    if FFN8:
                        w1t = wsp.tile([128, ND2, 2, 128], f8, tag="ws",
                                       name=f"w1_{l}_{f}")
                        nc.sync.dma_start(w1t[:], w1_d[l, f])
                        for j2 in range(ND2):
                            for c in range(2):
                                nc.tensor.matmul(
                                    gp[:, c, :],
                                    w1t[:, j2, :, :],
                                    h2T[:, 2 * j2:2 * j2 + 2,
                                        c * 512:(c + 1) * 512],
                                    start=(j2 == 0),
                                    stop=(j2 == ND2 - 1),
                                    perf_mode=DR,
                                )
                    else:
                        w1t = wsp.tile([128, ND, 128], f16, tag="ws",
                                       name=f"w1_{l}_{f}")
                        nc.sync.dma_start(w1t[:], w1_d[l, f])
                        for j in range(ND):
                            for c in range(2):
                                nc.tensor.matmul(
                                    gp[:, c, :],
                                    w1t[:, j, :],
                                    h2T[:, j, c * 512:(c + 1) * 512],
                                    start=(j == 0),
                                    stop=(j == ND - 1),
                                )
                    gsc = GDQ if FFN8 else 1.0
                    if has_b1:
                        nc.scalar.activation(
                            gq[f // 8][:, f % 8, 0:1024],
                            gp[:].rearrange("p c q -> p (c q)"),
                            AF.Gelu, bias=b1_sb[:, f:f + 1], scale=gsc)
                    else:
                        nc.scalar.activation(
                            gq[f // 8][:, f % 8, 0:1024],
                            gp[:].rearrange("p c q -> p (c q)"),
                            AF.Gelu, scale=gsc)

            ffn1_all()

            # GEMM2: acc pairs (2 t-tiles per 2-bank slot), w2 streamed
            last = nl - 1
            hT_next = None if l == last else htpool.tile(
                [128, ND, L], f8, tag="hT", name=f"hTn_{l}")
            hnbuf = {}

            def ffn2(tg):
                for c in range(2):
                    cs = slice(c * 512, (c + 1) * 512)
                    ys = [psB.tile([128, 2, 512], f32, tag="wk",
                                   name=f"psy_{l}_{c}_{tg[0]}_{k}")
                          for k in range((len(tg) + 1) // 2)]
                    if FFN8:
                        for f2 in range(NF2):
                            w2t = w2p.tile([128, 2, 512], f8, tag="w2s")
                            nc.sync.dma_start(w2t[:], w2_d[l, c, f2])
                            q, fo = f2 // 4, 2 * (f2 % 4)
                            for k, i in enumerate(tg):
                                nc.tensor.matmul(
                                    ys[k // 2][:, k % 2, :],
                                    gq[q][:, fo:fo + 2, i * 128:(i + 1) * 128],
                                    w2t[:],
                                    start=(f2 == 0),
                                    stop=(f2 == NF2 - 1),
                                    perf_mode=DR,
                                )
                        dq = 1.0 / SW2
                    else:
                        for f2 in range(NF2):
                            w2t = w2p.tile([128, 2, 512], f16, tag="w2s")
                            nc.sync.dma_start(w2t[:], w2_d[l, c, f2])
                            for ff in range(2):
                                f = 2 * f2 + ff
                                for k, i in enumerate(tg):
                                    nc.tensor.matmul(
                                        ys[k // 2][:, k % 2, :],
                                        gq[f // 8][:, f % 8,
                                           i * 128:(i + 1) * 128],
                                        w2t[:, ff, :],
                                        start=(f == 0),
                                        stop=(f == NF - 1),
                                    )
                        dq = 1.0
                    for k, i in enumerate(tg):
                        nc.vector.scalar_tensor_tensor(
                            x[:, i, cs], ys[k // 2][:, k % 2, :], dq,
                            x[:, i, cs], OP.mult, OP.add)
                        if has_b2:
                            nc.vector.tensor_add(x[:, i, cs], x[:, i, cs],
                                                 b2_sb[:, cs])

            epn = epsa
            ivn = 1.0 / (AH * AH)
            if l != last:
                ffn2((0, 1, 2, 3))
                ln_group(range(0, 4), hnbuf, epn, ivn, f"n0{l}")
                ffn2((4, 5))
                ln_group((4, 5), hnbuf, epn, ivn, f"n4{l}")
                lnT_pair(hnbuf, 0, hT_next)
                lnT_pair(hnbuf, 2, hT_next)
                ffn2((6, 7))
                ln_group((6, 7), hnbuf, epn, ivn, f"n6{l}")
                v_pending = big.tile([128, NT, 16 * 65], f8, tag="big",
                                     name=f"v{l+1}")
                # early V GEMM for next layer (tiles 0-3) while DVE does LN
                hT = hT_next
                v_gemm(range(0, 4), v_pending, wqv)
                lnT_pair(hnbuf, 4, hT_next)
                lnT_pair(hnbuf, 6, hT_next)
            else:
                # final layer: fuse the final LN + store into the FFN2 tail
                flns = f4k.tile([128, D], f32, tag="f4k")
                nc.sync.dma_start(flns[:], fs_d[:])
                flnb = f4k.tile([128, D], f32, tag="f4k")
                nc.sync.dma_start(flnb[:], fb_d[:])

                def fln_emit(tiles, suf):
                    fbuf = {}
                    ln_group(tiles, fbuf, epst, 1.0, suf)
                    for i in tiles:
                        xo = f4k.tile([128, D], f32, tag="xo", name=f"xo_{i}")
                        nc.vector.tensor_mul(xo[:], fbuf[i][:], flns[:])
                        nc.vector.tensor_add(xo[:], xo[:], flnb[:])
                        nc.sync.dma_start(out_d[i * 128:(i + 1) * 128, :], xo[:])

                ffn2((0, 1, 2, 3))
                fln_emit(range(0, 4), "f0")
                ffn2((4, 5))
                fln_emit((4, 5), "f4")
                ffn2((6, 7))
                fln_emit((6, 7), "f6")

    nc.compile()
    return nc


def _host_prep(sensor_tokens, traj_tokens, pos_embed, ln1_s, ln1_b,
               qkv_w, qkv_b, out_w, out_b, ln2_s, ln2_b,
               w1, b1, w2, b2, fln_s, fln_b, nl=NL):
    """Fold LN affine params into weights; retile + fp8-cast with scaling."""
    fp = np.float32
    x_all = np.concatenate([sensor_tokens, traj_tokens], axis=1).astype(fp)
    x_all = x_all + pos_embed[:L][None].astype(fp)

    wqk8 = np.empty((nl, 16, 128, ND2, 2, 128), F8NP)
    wqv8 = np.empty((nl, 128, ND, 1024), F8NP)
    bqh = np.empty((nl, 128, ND), fp)
    wo8 = np.empty((nl, 128, ND2, 2, 1024), F8NP)
    boh = np.empty((nl, 128, D), np.float16)
    if FFN8:
        w1T = np.empty((nl, NF, 128, ND2, 2, 128), F8NP)
        w2T = np.empty((nl, 2, NF2, 128, 2, 512), F8NP)
    else:
        w1T = np.empty((nl, NF, 128, ND, 128), np.float16)
        w2T = np.empty((nl, 2, NF2, 128, 2, 512), np.float16)
    b1h = np.empty((nl, 128, NF), fp)
    b2h = np.empty((nl, 128, D), np.float16)

    for i in range(nl):
        Wq = qkv_w[i].astype(fp)                                  # [3D, D]
        bfull = qkv_b[i].astype(fp) + Wq @ ln1_b[i].astype(fp)    # [3D]
        Wq = Wq * ln1_s[i].astype(fp)[None, :]
        WqT = (Wq.T * SW).astype(F8NP)                            # [D, 3D]
        qk = WqT[:, :2 * D].reshape(ND2, 2, 128, 16, 128)
        wqk8[i] = qk.transpose(3, 2, 0, 1, 4)
        wqv8[i] = WqT[:, 2 * D:].reshape(ND, 128, 1024).transpose(1, 0, 2)
        bqh[i] = bfull[:D].reshape(ND, 128).T
        bv = bfull[2 * D:]
        Wo = out_w[i].astype(fp)                                  # [D, D]
        bo = out_b[i].astype(fp) + Wo @ bv
        wo8[i] = (Wo.T * SW).astype(F8NP).reshape(ND2, 2, 128, 1024).transpose(2, 0, 1, 3)
        boh[i] = np.broadcast_to(bo.astype(np.float16), (128, D))
        W1 = w1[i].astype(fp)                                     # [FF, D]
        b1f = b1[i].astype(fp) + W1 @ ln2_b[i].astype(fp)
        W1 = W1 * ln2_s[i].astype(fp)[None, :]
        if FFN8:
            W1t = (W1.T * SW1).astype(F8NP)                       # [D, FF]
            # [d, ff] -> (f, p(ffchunk? no: p=d%128), j2, jj, col)
            w1T[i] = W1t.reshape(ND2, 2, 128, NF, 128).transpose(3, 2, 0, 1, 4)
            W2t = (w2[i].astype(fp).T * SW2).astype(F8NP)         # [FF, D]
            w2T[i] = W2t.reshape(NF2, 2, 128, 2, 512).transpose(3, 0, 2, 1, 4)
        else:
            W1t = W1.T.astype(np.float16)                         # [D, FF]
            w1T[i] = W1t.reshape(ND, 128, NF, 128).transpose(2, 1, 0, 3)
            W2t = w2[i].astype(fp).T.astype(np.float16)           # [FF, D]
            w2T[i] = W2t.reshape(NF2, 2, 128, 2, 512).transpose(3, 0, 2, 1, 4)
        b1h[i] = b1f.reshape(NF, 128).T
        b2h[i] = np.broadcast_to(b2[i].astype(np.float16), (128, D))

    common = dict(
        wqk=wqk8, wqv=wqv8, bq=bqh, wo=wo8, bo=boh,
        w1=w1T, b1=b1h, w2=w2T, b2=b2h,
        flns=np.broadcast_to(fln_s.astype(fp), (128, D)).copy(),
        flnb=np.broadcast_to(fln_b.astype(fp), (128, D)).copy(),
        ident=np.eye(128, dtype=np.float16),
        mbias=np.triu(np.full((128, 128), -30000.0, np.float16), 1),
    )
    in_maps = [dict(common, x0=np.ascontiguousarray(x_all[c])) for c in range(B)]
    return in_maps


_NC = {}
LAST_RESULT = None


def kernel(**inputs):
    global LAST_RESULT
    in_maps = _host_prep(**inputs)
    m0 = in_maps[0]
    flags = (bool(np.any(m0["bq"])), bool(np.any(m0["bo"])),
             bool(np.any(m0["b1"])), bool(np.any(m0["b2"])))
    if flags not in _NC:
        _NC[flags] = build_nc(NL, *flags)
    res = run_bass_kernel_spmd(_NC[flags], in_maps, core_ids=list(range(B)))
    LAST_RESULT = res
    return np.stack([res.results[c]["out"] for c in range(B)]).astype(np.float32)
